# revision 69
# baseline (speedup 1.0000x reference)
"""Trainium2 Bass kernel for nn_BAC_15152644620305.

Per batch element (1 per NeuronCore, 8 cores):
  p_dense = relu(p @ W1 + b1); q_dense = relu(q @ W2 + b2)
  A = (p_dense @ q_dense.T) / sqrt(600)
  passage_aligned = softmax_rows(A) @ passage ; query_aligned = softmax_cols(A).T @ query
  6 factorization-machine heads on {concat, diff, mul} pairs -> [L, 3] x 2 outputs.

Implementation notes:
  - All heavy matmuls in bf16 (1 cyc/row on PE), fp32 PSUM accumulation;
    aligned/affinity contractions use fp8e4m3 DoubleRow (0.5 cyc/row).
  - Inputs arrive pre-cast to bf16 (halves input DMA, no on-chip f32 casts);
    dense weights/stationaries pre-packed bf16 on the host.
  - Affinity computed ONCE; exp(A) (fp8) is transposed into exp(A)^T by xbar
    DMA transposes (uint16 views batched 8 blocks/instruction) on the
    otherwise-idle DMA engines -- no second affinity+exp pass.  The paT
    contraction consumes the byte-transposed layout via adjacent-(q,q+1)
    DoubleRow pairs; its stationary natural tiles are built adjacent-paired
    directly by a row-interleaved second DMA load of the passage tensor.
  - Affinity is emitted as a wavefront inside the transpose/dense phase:
    each (row-pair, col-chunk) fires as soon as its dense outputs exist,
    spreading the exp (ACT) load into the DMA-paced start.
  - exp without max-subtraction (affinity values are in [0.1, 1.1]).
  - Softmax denominators ride along as an extra ones-column in the aligned
    matmuls' stationary operand, landing at an aligned output partition (96).
  - FM heads algebraically reduced: the x^2 @ V^2.T term needs only
    sum_k V_k^2; diff projections are linear combos of the qa/p projections;
    per-head combination is one small stationary matmul per output chunk.
    The two independent elementwise squares run on ACT (idle during FM),
    the product chain on DVE; S-builds run right after the projections so
    all PSUM banks free before the next aligned pass (its matmuls overlap
    the builds), and the combine matmuls+outputs interleave into that pass.
"""
import numpy as np

L_FULL = 2048
D = 600
U = 300
KFM = 5
N_CORES = 8
SCALE = float(1.0 / np.sqrt(np.float32(D)))

DCH = [(0, 128), (128, 128), (256, 128), (384, 128), (512, 88)]   # D chunks
UCH = [(0, 128), (128, 128), (256, 44)]                           # U chunks
ONES_COL = 608        # column in the 640-wide natural tile holding the ones
ONES_ROW = 96         # output partition where the denominator row lands
NATW = 640


def _emit(nc, L):
    import concourse.bass as bass
    import concourse.mybir as mybir
    import concourse.tile as tile
    from concourse.masks import make_identity
    from contextlib import ExitStack

    f32 = mybir.dt.float32
    bf16 = mybir.dt.bfloat16
    fp8 = mybir.dt.float8e4
    AF = mybir.ActivationFunctionType
    ds = bass.ds

    LT = L // 128               # l tiles
    NCW = min(512, L)           # moving-dim chunk width
    NCX = L // NCW              # chunks per L
    TG = 4 if LT % 4 == 0 else 1  # l-tiles per transpose psum batch

    x_d = nc.dram_tensor("x", [2, L, D], bf16, kind="ExternalInput")
    wp_d = nc.dram_tensor("wpack", [10, 128, U], bf16, kind="ExternalInput")
    sp_d = nc.dram_tensor("statp", [10, 128, 36], bf16, kind="ExternalInput")
    c2_d = nc.dram_tensor("comb2", [128, 6], bf16, kind="ExternalInput")
    bp_d = nc.dram_tensor("biasp", [128, 6], f32, kind="ExternalInput")
    w0_d = nc.dram_tensor("w0col", [3, 2], f32, kind="ExternalInput")
    out_d = nc.dram_tensor("out", [2, 3, L], f32, kind="ExternalOutput")

    u16 = mybir.dt.uint16

    with tile.TileContext(nc) as tc, ExitStack() as ctx:
        const = ctx.enter_context(tc.tile_pool(name="const", bufs=1))
        big = ctx.enter_context(tc.tile_pool(name="big", bufs=1))
        epool = ctx.enter_context(tc.tile_pool(name="epool", bufs=LT // 2))
        natp = ctx.enter_context(tc.tile_pool(name="natp", bufs=LT))
        nf32p = ctx.enter_context(tc.tile_pool(name="nf32p", bufs=6))
        fmt = ctx.enter_context(tc.tile_pool(name="fmt", bufs=4))
        sp = ctx.enter_context(tc.tile_pool(name="sp", bufs=2))
        rp = ctx.enter_context(tc.tile_pool(name="rp", bufs=2))
        ob = ctx.enter_context(tc.tile_pool(name="ob", bufs=1))
        ps = ctx.enter_context(tc.tile_pool(name="ps", bufs=8, space="PSUM"))

        def pst(p_cnt=128, w=NCW):
            return ps.tile([p_cnt, w], f32, tag="ps", name="pst")

        # ------- constants (packed loads on the scalar HWDGE queue) -------
        identb = const.tile([128, 128], bf16, tag="identb")
        make_identity(nc, identb)
        onesb = const.tile([128, 128], bf16, tag="onesb")
        nc.vector.memset(onesb[:], 1.0)
        w0sb = const.tile([3, 2], f32, tag="w0sb")
        nc.scalar.dma_start(w0sb[:], w0_d[:])

        # weights / stationaries arrive pre-packed as bf16: straight DMA loads
        Wall = const.tile([128, 10 * U], bf16, tag="Wall")
        nc.scalar.dma_start(
            Wall[:].rearrange("p (t c) -> p t c", t=10),
            wp_d[:].rearrange("t p c -> p t c"))
        Wsb = [[Wall[:, ds((t * 5 + k) * U, U)] for k in range(5)]
               for t in range(2)]

        Sall = const.tile([128, 360], bf16, tag="Sall")
        nc.scalar.dma_start(
            Sall[:].rearrange("p (t c) -> p t c", t=10),
            sp_d[:].rearrange("t p c -> p t c"))
        stat = [[Sall[:, ds((s * 5 + k) * 36, 36)] for k in range(5)]
                for s in range(2)]

        cb2 = const.tile([128, 6], bf16, tag="cb2")
        nc.scalar.dma_start(cb2[:], c2_d[:])

        bsb = const.tile([128, 6], f32, tag="bsb")
        nc.scalar.dma_start(bsb[:], bp_d[:])

        # ---------------- phase 1: transpose inputs -> pT/qT (bf16 [d, L]) ----
        xT = [[], []]
        for t in range(2):
            for k in range(len(DCH)):
                xT[t].append(big.tile([128, L], bf16, tag=f"xT{t}_{k}",
                                      name=f"xT{t}_{k}"))
        # phase 1+2 interleaved per l-group: transpose inputs -> pT/qT, then
        # the dense matmuls for that group's columns (keeps PE fed during the
        # next group's DMA + cast)
        # u-chunks 0,1 live as one fp8 PAIR tile (DoubleRow operand for the
        # affinity matmuls); the 44-row chunk 2 stays bf16 (base-0 + base-64)
        dTP = [big.tile([128, 2, L], fp8, tag=f"dTP{t}", name=f"dTP{t}")
               for t in range(2)]
        dT2 = [big.tile([128, L], fp8, tag=f"dT2{t}", name=f"dT2{t}")
               for t in range(2)]
        # E2 = byte-transposed E1 (exp(A)^T), written by DMA xbar transposes.
        # Layout [r, g, p, s]: q-row 256*g + 2*r + s, p-col p (adjacent-pair
        # DoubleRow convention over q).
        e2all = big.tile([128, LT // 2, L, 2], fp8, tag="e2all", name="e2all")
        # nats[1] (query): standard pairing (a, j) <-> p-row 256*pi + 128*j + a
        # (matches E1's exp-written slot layout, contraction over p).
        # nats[0] (passage): ADJACENT pairing (r, s) <-> q-row 256*pi + 2*r + s
        # (matches e2all, contraction over q), built from a row-interleaved
        # second load of the passage tensor.
        nats = [[None] * (LT // 2) for _ in range(2)]
        x0i = x_d[0].rearrange("(g r s) d -> g r s d", r=128, s=2)

        DRm = mybir.MatmulPerfMode.DoubleRow
        E1 = [None] * (LT // 2)

        def aff_chunk(pi2, nx):
            """Affinity rows 256*pi2..+256 x cols nx*NCW..+NCW -> exp -> E1."""
            if E1[pi2] is None:
                E1[pi2] = epool.tile([128, 2, L], fp8, tag="E",
                                     name=f"E1_{pi2}")
            e = E1[pi2]
            nsl = ds(nx * NCW, NCW)
            accs = (pst(), pst())
            for j in (0, 1):
                isl = ds((2 * pi2 + j) * 128, 128)
                # u-chunks 0+1 in one fp8 DoubleRow pass
                nc.tensor.matmul(accs[j][:, :], dTP[0][:, :, isl],
                                 dTP[1][:, :, nsl],
                                 start=True, stop=False, perf_mode=DRm)
            # 44-row K chunk: the pair's two matmuls go to disjoint PE
            # row-groups and run concurrently
            nc.tensor.matmul(accs[0][:, :],
                             dT2[0][0:44, ds(2 * pi2 * 128, 128)],
                             dT2[1][0:44, nsl],
                             start=False, stop=True, tile_position=(0, 0))
            nc.tensor.matmul(accs[1][:, :],
                             dT2[0][64:108, ds((2 * pi2 + 1) * 128, 128)],
                             dT2[1][64:108, nsl],
                             start=False, stop=True, tile_position=(64, 0))
            for j in (0, 1):
                nc.scalar.activation(e[:, j, nsl], accs[j][:, :],
                                     AF.Exp, scale=SCALE)

        for g in range(LT // TG):
            gw = TG * 128
            for pi in range(g * TG // 2, (g + 1) * TG // 2):
                nf2 = nf32p.tile([128, 2, D], bf16, tag="nf2", name="nf2",
                                 bufs=1)
                nc.sync.dma_start(nf2[:], x0i[pi])
                nt0 = natp.tile([128, 2, NATW], fp8, tag="nat",
                                name=f"nat0_{pi}")
                nats[0][pi] = nt0
                nc.gpsimd.memset(nt0[:, :, D:NATW], 0.0)
                nc.gpsimd.memset(nt0[:, :, ONES_COL:ONES_COL + 1], 1.0)
                nc.vector.tensor_copy(nt0[:, :, 0:D], nf2[:])
            for t in range(2):
                # 2 d-chunks per bf16 psum tile (same 2KB bank footprint as
                # one f32 slot) -> 3 slots instead of 5, more slot headroom
                # for the dense accumulators and the next group's transposes
                pjs2 = [ps.tile([128, 2 * NCW], bf16, tag="ps", name="pjs")
                        for _ in range((len(DCH) + 1) // 2)]
                pjs = [pjs2[k // 2][:, ds((k % 2) * NCW, NCW)]
                       for k in range(len(DCH))]
                for ii in range(TG):
                    i = g * TG + ii
                    nf = nf32p.tile([128, D], bf16, tag="nf", name="nf",
                                    bufs=4)
                    eng = nc.sync if (g == 0 or i % 2 == 0) else nc.scalar
                    eng.dma_start(nf[:], x_d[t, ds(i * 128, 128), :])
                    nfb = nf
                    # build the fp8 natural-layout pair tile (DoubleRow operand
                    # of the aligned matmuls) from the same load; only t=1 --
                    # t=0 is built adjacent-paired from the nf2 loads above
                    pi, j = i // 2, i % 2
                    if t == 1:
                        if j == 0:
                            nats[1][pi] = natp.tile([128, 2, NATW], fp8,
                                                    tag="nat",
                                                    name=f"nat1_{pi}")
                            nc.gpsimd.memset(nats[1][pi][:, :, D:NATW], 0.0)
                            nc.gpsimd.memset(
                                nats[1][pi][:, :, ONES_COL:ONES_COL + 1], 1.0)
                        nt = nats[1][pi]
                        # DVE/Pool: ACT is the bottleneck of this merged phase
                        if j == 0:
                            nc.vector.tensor_copy(nt[:, j, 0:D], nf[:])
                        else:
                            nc.gpsimd.tensor_copy(nt[:, j, 0:D], nf[:])
                    for k, (doff, dcnt) in enumerate(DCH):
                        nc.tensor.transpose(
                            pjs[k][:dcnt, ds(ii * 128, 128)],
                            nfb[:, ds(doff, dcnt)], identb[:])
                for k, (doff, dcnt) in enumerate(DCH):
                    # all on DVE: ACT is the bottleneck of this merged phase
                    nc.vector.tensor_copy(xT[t][k][:dcnt, ds(g * gw, gw)],
                                          pjs[k][:dcnt, ds(0, gw)])
            if gw == NCW:
                for t in range(2):
                    for m, (uoff, ucnt) in enumerate(UCH[:2]):
                        acc = pst()
                        for k, (doff, dcnt) in enumerate(DCH):
                            nc.tensor.matmul(
                                acc[:ucnt, :],
                                Wsb[t][k][:dcnt, ds(uoff, ucnt)],
                                xT[t][k][:dcnt, ds(g * NCW, NCW)],
                                start=(k == 0), stop=(k == len(DCH) - 1))
                        if g >= 2:
                            # late groups: ACT is saturated by the affinity
                            # exp wavefront; fused add+max on the idler DVE
                            nc.vector.tensor_scalar(
                                dTP[t][:, m, ds(g * NCW, NCW)], acc[:ucnt, :],
                                bsb[:ucnt, t * 3 + m: t * 3 + m + 1], 0.0,
                                mybir.AluOpType.add, mybir.AluOpType.max)
                        else:
                            nc.scalar.activation(
                                dTP[t][:, m, ds(g * NCW, NCW)], acc[:ucnt, :],
                                AF.Relu,
                                bias=bsb[:ucnt, t * 3 + m: t * 3 + m + 1])
                # the 44-row M-chunk: both tensors' matmuls in concurrent
                # col-groups (0 and 64) of one psum tile
                uoff, ucnt = UCH[2]
                acc2 = pst()
                for k, (doff, dcnt) in enumerate(DCH):
                    fl = (k == 0, k == len(DCH) - 1)
                    nc.tensor.matmul(
                        acc2[0:ucnt, :],
                        Wsb[0][k][:dcnt, ds(uoff, ucnt)],
                        xT[0][k][:dcnt, ds(g * NCW, NCW)],
                        start=fl[0], stop=fl[1], tile_position=(0, 0),
                        skip_group_check=True)
                    nc.tensor.matmul(
                        acc2[64:64 + ucnt, :],
                        Wsb[1][k][:dcnt, ds(uoff, ucnt)],
                        xT[1][k][:dcnt, ds(g * NCW, NCW)],
                        start=fl[0], stop=fl[1], tile_position=(0, 64),
                        skip_group_check=True)
                for t in range(2):
                    pb = t * 64
                    for dst in (0, 64):
                        # evict to base 0 (affinity k2 slice) and base 64
                        # (its row-pair partner slice)
                        if g >= 2:
                            nc.vector.tensor_scalar(
                                dT2[t][dst:dst + ucnt, ds(g * NCW, NCW)],
                                acc2[pb:pb + ucnt, :],
                                bsb[:ucnt, t * 3 + 2: t * 3 + 3], 0.0,
                                mybir.AluOpType.add, mybir.AluOpType.max)
                        else:
                            nc.scalar.activation(
                                dT2[t][dst:dst + ucnt, ds(g * NCW, NCW)],
                                acc2[pb:pb + ucnt, :], AF.Relu,
                                bias=bsb[:ucnt, t * 3 + 2: t * 3 + 3])
                # wavefront affinity: every (row-pair, col-chunk) whose dense
                # inputs just became ready -- spreads the exp (ACT) load into
                # the DMA-paced transpose/dense phase
                for pi2 in range(2 * g, 2 * g + 2):
                    for nx in range(g + 1):
                        aff_chunk(pi2, nx)
                for pi2 in range(0, 2 * g):
                    aff_chunk(pi2, g)
        if TG * 128 != NCW:
            for t in range(2):
                for m, (uoff, ucnt) in enumerate(UCH):
                    for nx in range(NCX):
                        acc = pst()
                        for k, (doff, dcnt) in enumerate(DCH):
                            nc.tensor.matmul(
                                acc[:ucnt, :],
                                Wsb[t][k][:dcnt, ds(uoff, ucnt)],
                                xT[t][k][:dcnt, ds(nx * NCW, NCW)],
                                start=(k == 0), stop=(k == len(DCH) - 1))
                        if m < 2:
                            nc.scalar.activation(
                                dTP[t][:, m, ds(nx * NCW, NCW)], acc[:ucnt, :],
                                AF.Relu,
                                bias=bsb[:ucnt, t * 3 + m: t * 3 + m + 1])
                        else:
                            for dst in (0, 64):
                                nc.scalar.activation(
                                    dT2[t][dst:dst + ucnt, ds(nx * NCW, NCW)],
                                    acc[:ucnt, :], AF.Relu,
                                    bias=bsb[:ucnt, t * 3 + m: t * 3 + m + 1])

        # E2: as each E1 pair tile completes, two xbar DMA transposes copy it
        # (viewed as uint16 q-pairs) into e2all, building exp(A)^T on the idle
        # DMA engines instead of a second affinity+exp pass.
        for pi in range(LT // 2):
            for j in range(2):
                inap = E1[pi][:, j, :].bitcast(u16).rearrange(
                    "p (g u) -> p g u", g=LT // 2)
                pc = (2 * pi + j) * 128
                outap = e2all[:, :, ds(pc, 128), :].bitcast(u16).squeeze()
                nc.sync.dma_start_transpose(outap, inap)

        # helpers ------------------------------------------------------------
        def aligned_T(nats, mov, side_tag, hooks=()):
            """alT tiles [d,L] bf16 = normalized aligned.T, via ones-row trick.

            mov(pi, nsl) yields the fp8 DoubleRow moving AP [128, 2, |nsl|];
            the stationary nats[pi] must pair contraction rows the same way.
            """
            alT = [big.tile([128, L], bf16, tag=f"alT{k}", name=f"alT{side_tag}{k}")
                   for k in range(len(DCH))]
            R = big.tile([128, L], bf16, tag="R", name=f"R{side_tag}")
            NP = LT // 2
            DR = mybir.MatmulPerfMode.DoubleRow
            hooks = list(hooks)
            if hooks:
                hooks.pop(0)()
            # pass A: last d-chunk (88 rows) + ones row at partition 96
            ps4 = [pst() for _ in range(NCX)]
            for pi in range(NP):
                for nx in range(NCX):
                    nc.tensor.matmul(ps4[nx][:, :],
                                     nats[pi][:, :, ds(512, 128)],
                                     mov(pi, ds(nx * NCW, NCW)),
                                     start=(pi == 0), stop=(pi == NP - 1),
                                     perf_mode=DR)
            for nx in range(NCX):
                rrb = rp.tile([128, NCW], bf16, tag="rrb", name="rrb")
                with nc.allow_low_precision(reason="R is consumed as bf16"):
                    nc.vector.reciprocal(rrb[ONES_ROW:ONES_ROW + 1, :],
                                         ps4[nx][ONES_ROW:ONES_ROW + 1, :])
                bc = pst()
                nc.tensor.matmul(bc[:, :], onesb[ONES_ROW:ONES_ROW + 1, 0:128],
                                 rrb[ONES_ROW:ONES_ROW + 1, :],
                                 start=True, stop=True,
                                 tile_position=(ONES_ROW, 0))
                nc.scalar.copy(R[:, ds(nx * NCW, NCW)], bc[:, :])
                nc.vector.tensor_mul(alT[4][0:88, ds(nx * NCW, NCW)],
                                     ps4[nx][0:88, :],
                                     R[0:88, ds(nx * NCW, NCW)])
            # passes B, C: d-chunks 0..3, two at a time
            for mm0 in (0, 2):
                if hooks:
                    hooks.pop(0)()
                accs = {}
                for m in (mm0, mm0 + 1):
                    for nx in range(NCX):
                        accs[(m, nx)] = pst()
                for pi in range(NP):
                    for m in (mm0, mm0 + 1):
                        for nx in range(NCX):
                            nc.tensor.matmul(accs[(m, nx)][:, :],
                                             nats[pi][:, :, ds(m * 128, 128)],
                                             mov(pi, ds(nx * NCW, NCW)),
                                             start=(pi == 0),
                                             stop=(pi == NP - 1),
                                             perf_mode=DR)
                for m in (mm0, mm0 + 1):
                    for nx in range(NCX):
                        nc.vector.tensor_mul(alT[m][:, ds(nx * NCW, NCW)],
                                             accs[(m, nx)][:, :],
                                             R[:, ds(nx * NCW, NCW)])
            while hooks:
                hooks.pop(0)()
            return alT, R

        def fm_proj(s, xTs, bTs):
            """FM projection matmuls for one side; returns live PSUM groups.

            d-chunk-outer loop: the elementwise temps are built full-width once
            per chunk (fewer DVE ops, deeper PE overlap); all four N-chunks'
            projection groups accumulate simultaneously (8 PSUM banks).
            """
            P1s = [ps.tile([128, NCW], f32, tag="ps", name="P1")
                   for _ in range(NCX)]
            P2s = [ps.tile([128, NCW], f32, tag="ps", name="P2")
                   for _ in range(NCX)]
            nk = len(DCH)
            for k, (doff, dcnt) in enumerate(DCH):
                x_fl = xTs[k][:dcnt, :]
                b_fl = bTs[k][:dcnt, :]
                tx2 = fmt.tile([128, L], bf16, tag="fmt", name="tx2")
                tb2 = fmt.tile([128, L], bf16, tag="fmt", name="tb2")
                txm = fmt.tile([128, L], bf16, tag="fmt", name="txm")
                txm2 = fmt.tile([128, L], bf16, tag="fmt", name="txm2")
                # the two independent squares go to ACT (idle through the FM
                # phases); the txm -> txm2 chain stays on the faster DVE
                nc.scalar.activation(tx2[:dcnt, :], x_fl, AF.Square)
                nc.scalar.activation(tb2[:dcnt, :], b_fl, AF.Square)
                nc.vector.tensor_mul(txm[:dcnt, :], x_fl, b_fl)
                nc.vector.tensor_mul(txm2[:dcnt, :], txm[:dcnt, :],
                                     txm[:dcnt, :])
                st = stat[s][k]
                fl = (k == 0, k == nk - 1)
                for nx in range(NCX):
                    nsl = ds(nx * NCW, NCW)
                    P1, P2 = P1s[nx], P2s[nx]
                    nc.tensor.matmul(P1[0:12, :], st[:dcnt, 0:12],
                                     xTs[k][:dcnt, nsl],
                                     start=fl[0], stop=fl[1],
                                     tile_position=(0, 0),
                                     skip_group_check=True)
                    nc.tensor.matmul(P1[32:44, :], st[:dcnt, 12:24],
                                     bTs[k][:dcnt, nsl],
                                     start=fl[0], stop=fl[1],
                                     tile_position=(0, 32),
                                     skip_group_check=True)
                    nc.tensor.matmul(P1[64:65, :], st[:dcnt, 35:36],
                                     txm2[:dcnt, nsl], start=fl[0], stop=fl[1],
                                     tile_position=(0, 64),
                                     skip_group_check=True)
                    nc.tensor.matmul(P2[0:2, :], st[:dcnt, 24:26],
                                     tx2[:dcnt, nsl], start=fl[0], stop=fl[1],
                                     tile_position=(0, 0),
                                     skip_group_check=True)
                    nc.tensor.matmul(P2[32:34, :], st[:dcnt, 26:28],
                                     tb2[:dcnt, nsl], start=fl[0], stop=fl[1],
                                     tile_position=(0, 32),
                                     skip_group_check=True)
                    nc.tensor.matmul(P2[64:71, :], st[:dcnt, 28:35],
                                     txm[:dcnt, nsl], start=fl[0], stop=fl[1],
                                     tile_position=(0, 64),
                                     skip_group_check=True)
            return P1s, P2s

        def fm_comb_build(P1s, P2s, nx, R):
            """S-build (ACT/DVE only) for one N-chunk: evict + square the FM
            groups into S1/S2. Frees the psum banks early so the next
            aligned_T pass's matmuls overlap with the builds."""
            P1, P2 = P1s[nx], P2s[nx]
            S1 = sp.tile([128, NCW], bf16, tag="S1", name="S1", bufs=4)
            S2 = sp.tile([128, NCW], bf16, tag="S2", name="S2", bufs=4)
            nc.gpsimd.memset(S1[:], 0.0)
            nc.gpsimd.memset(S2[:], 0.0)
            # split evictions ACT/DVE so the S-build runs in parallel
            nc.scalar.copy(S1[0:12, :], P1[0:12, :])
            nc.scalar.copy(S1[32:44, :], P1[32:44, :])
            nc.vector.tensor_copy(S1[64:66, :], P2[0:2, :])
            nc.vector.tensor_copy(S1[96:98, :], P2[32:34, :])
            nc.vector.tensor_copy(S2[0:7, :], P2[64:71, :])
            nc.vector.tensor_copy(S2[32:33, :], P1[64:65, :])
            # B-group Vd columns carry -Vd, so diff quads are also an add.
            TA = sp.tile([10, NCW], f32, tag="TA", name="TA", bufs=2)
            nc.vector.tensor_add(TA[0:10, :], P1[0:10, :], S1[32:42, :])
            nc.scalar.activation(S2[64:74, :], TA[:, :], AF.Square)
            nc.scalar.activation(S2[96:101, :], S2[0:5, :], AF.Square)
            return S1, S2

        def fm_comb_mm(s, Ss, nx):
            """Combine matmuls + bias + output DMA for one N-chunk."""
            S1, S2 = Ss[nx]
            nsl = ds(nx * NCW, NCW)
            cps = ps.tile([3, NCW], f32, tag="ps", name="cps")
            nc.tensor.matmul(cps[:, :], cb2[0:98, 0:3], S1[0:98, :],
                             start=True, stop=False)
            nc.tensor.matmul(cps[:, :], cb2[0:101, 3:6], S2[0:101, :],
                             start=False, stop=True)
            o = ob.tile([3, NCW], f32, tag="ob", name="o")
            nc.scalar.activation(o[:, :], cps[:, :], AF.Identity,
                                 bias=w0sb[:, s:s + 1])
            nc.sync.dma_start(out_d[s, :, nsl], o[:, :])

        # ---------------- main flow ----------------
        qaT, Rq = aligned_T(nats[1],
                            lambda pi, nsl: E1[pi][:, :, nsl], "q")
        P1s, P2s = fm_proj(0, qaT, xT[0])         # passage-side projections
        # S-builds (ACT/DVE) free all 8 psum banks up front; the combine
        # matmuls + outputs interleave with paT's passes so PE never stalls
        Ss0 = [fm_comb_build(P1s, P2s, nx, Rq) for nx in range(NCX)]
        # passage_aligned.T from the DMA-transposed exp(A)^T (adjacent q-pairs)
        paT, Rp = aligned_T(nats[0],
                            lambda pi, nsl: e2all[:, pi, nsl, :].rearrange(
                                "p n s -> p s n"), "p",
                            hooks=[lambda nx=nx: fm_comb_mm(0, Ss0, nx)
                                   for nx in range(NCX)])
        P1s1, P2s1 = fm_proj(1, paT, xT[1])       # query-side projections
        # tail: interleave build/mm per N-chunk so each combine's matmuls
        # start as soon as its own S-build is done (nothing follows to
        # overlap with, so serialization here is pure wall time)
        Ss1 = []
        for nx in range(NCX):
            Ss1.append(fm_comb_build(P1s1, P2s1, nx, Rp))
            fm_comb_mm(1, Ss1, nx)


def _host_prep(W1, b1, W2, b2, cat_w0, cat_w, cat_V, dm_w0, dm_w, dm_V):
    stat = np.zeros((2, D, 36), np.float32)
    for s in range(2):
        ci, di, mi = s, s, s + 2
        Va = cat_V[ci][:, :D]
        Vb = cat_V[ci][:, D:]
        Vd = dm_V[di]
        Vm = dm_V[mi]
        stat[s, :, 0:5] = Va.T
        stat[s, :, 5:10] = Vd.T
        stat[s, :, 10] = cat_w[ci, :D]
        stat[s, :, 11] = dm_w[di]
        stat[s, :, 12:17] = Vb.T
        stat[s, :, 17:22] = -Vd.T   # negated: quad build is then a single add
        stat[s, :, 22] = cat_w[ci, D:]
        stat[s, :, 23] = dm_w[di]
        stat[s, :, 24] = (Va ** 2).sum(0)
        stat[s, :, 25] = (Vd ** 2).sum(0)
        stat[s, :, 26] = (Vb ** 2).sum(0)
        stat[s, :, 27] = (Vd ** 2).sum(0)
        stat[s, :, 28:33] = Vm.T
        stat[s, :, 33] = dm_w[mi]
        stat[s, :, 34] = (Vd ** 2).sum(0)
        stat[s, :, 35] = (Vm ** 2).sum(0)

    # packed combine matrices: S1 = [X@0, B@32, X2@64, B2@96],
    # S2 = [M@0, M2@32, TQ@64, TQM@96]
    comb2 = np.zeros((128, 6), np.float32)
    C1, C2 = comb2[:, 0:3], comb2[:, 3:6]
    C1[10, 0] = 1.0     # x@w_cat -> c_cat
    C1[11, 1] = 1.0     # x@w_d -> c_diff
    C1[32 + 10, 0] = 1.0
    C1[32 + 11, 1] = -1.0
    C1[64, 0] = -0.5    # x2@u_cat
    C1[65, 1] = -0.5    # x2@u_d
    C1[96, 0] = -0.5    # b2@u_cat
    C1[97, 1] = -0.5    # b2@u_d
    C2[5, 2] = 1.0      # mul@w_m
    C2[6, 1] = 1.0      # mul@u_d (from -0.5 * -2)
    C2[32, 2] = -0.5    # mul2@u_m
    C2[64:69, 0] = 0.5  # cat quads
    C2[69:74, 1] = 0.5  # diff quads
    C2[96:101, 2] = 0.5  # mul quads

    # packed per-d-chunk weights / stationaries; the dense bias rides as an
    # extra stationary row (matched by a ones row in xT's last chunk)
    wpack = np.zeros((10, 128, U), np.float32)
    statp = np.zeros((10, 128, 36), np.float32)
    for t, W in enumerate((W1, W2)):
        for k, (doff, dcnt) in enumerate(DCH):
            wpack[t * 5 + k, :dcnt] = W[doff:doff + dcnt]
    for s in range(2):
        for k, (doff, dcnt) in enumerate(DCH):
            statp[s * 5 + k, :dcnt] = stat[s, doff:doff + dcnt]

    biasp = np.zeros((128, 6), np.float32)
    for t, b in enumerate((b1, b2)):
        for m, (uoff, ucnt) in enumerate(UCH):
            biasp[:ucnt, t * 3 + m] = b[uoff:uoff + ucnt]

    w0col = np.zeros((3, 2), np.float32)
    for s in range(2):
        w0col[0, s] = cat_w0[s, 0]
        w0col[1, s] = dm_w0[s, 0]
        w0col[2, s] = dm_w0[s + 2, 0]
    return wpack, statp, comb2, biasp, w0col


_PROG = None


def _get_prog():
    global _PROG
    if _PROG is None:
        from concourse import bacc
        nc = bacc.Bacc(None, target_bir_lowering=False)
        _emit(nc, L_FULL)
        nc.finalize()
        _PROG = nc
    return _PROG


def _in_maps(stack_input, W1, b1, W2, b2, fm_cat_w0, fm_cat_w, fm_cat_V,
             fm_dm_w0, fm_dm_w, fm_dm_V):
    import ml_dtypes
    f = lambda a: np.ascontiguousarray(np.asarray(a, np.float32))
    bf = lambda a: np.ascontiguousarray(np.asarray(a, ml_dtypes.bfloat16))
    stack_input = bf(stack_input)
    wpack, statp, comb2, biasp, w0col = _host_prep(
        f(W1), f(b1), f(W2), f(b2), f(fm_cat_w0), f(fm_cat_w), f(fm_cat_V),
        f(fm_dm_w0), f(fm_dm_w), f(fm_dm_V))
    common = {"wpack": bf(wpack), "statp": bf(statp), "comb2": bf(comb2),
              "biasp": biasp, "w0col": w0col}
    return [dict(common, x=np.ascontiguousarray(stack_input[:, b]))
            for b in range(N_CORES)]


def kernel(stack_input, W1, b1, W2, b2, fm_cat_w0, fm_cat_w, fm_cat_V,
           fm_dm_w0, fm_dm_w, fm_dm_V):
    from concourse.bass_utils import run_bass_kernel_spmd

    in_maps = _in_maps(stack_input, W1, b1, W2, b2, fm_cat_w0, fm_cat_w,
                       fm_cat_V, fm_dm_w0, fm_dm_w, fm_dm_V)
    nc = _get_prog()
    res = run_bass_kernel_spmd(nc, in_maps, core_ids=list(range(N_CORES)))
    outs = [r["out"] for r in res.results]            # each [2, 3, L]
    fp = np.stack([o[0].T for o in outs]).astype(np.float32)   # [8, L, 3]
    fq = np.stack([o[1].T for o in outs]).astype(np.float32)
    return fp, fq



# revision 70
# speedup vs baseline: 1.0016x; 1.0016x over previous
"""Trainium2 Bass kernel for nn_BAC_15152644620305.

Per batch element (1 per NeuronCore, 8 cores):
  p_dense = relu(p @ W1 + b1); q_dense = relu(q @ W2 + b2)
  A = (p_dense @ q_dense.T) / sqrt(600)
  passage_aligned = softmax_rows(A) @ passage ; query_aligned = softmax_cols(A).T @ query
  6 factorization-machine heads on {concat, diff, mul} pairs -> [L, 3] x 2 outputs.

Implementation notes:
  - All heavy matmuls in bf16 (1 cyc/row on PE), fp32 PSUM accumulation;
    aligned/affinity contractions use fp8e4m3 DoubleRow (0.5 cyc/row).
  - Inputs arrive pre-cast to bf16 (halves input DMA, no on-chip f32 casts);
    dense weights/stationaries pre-packed bf16 on the host.
  - Affinity computed ONCE; exp(A) (fp8) is transposed into exp(A)^T by xbar
    DMA transposes (uint16 views batched 8 blocks/instruction) on the
    otherwise-idle DMA engines -- no second affinity+exp pass.  The paT
    contraction consumes the byte-transposed layout via adjacent-(q,q+1)
    DoubleRow pairs; its stationary natural tiles are built adjacent-paired
    directly by a row-interleaved second DMA load of the passage tensor.
  - Affinity is emitted as a wavefront inside the transpose/dense phase:
    each (row-pair, col-chunk) fires as soon as its dense outputs exist,
    spreading the exp (ACT) load into the DMA-paced start.
  - exp without max-subtraction (affinity values are in [0.1, 1.1]).
  - Softmax denominators ride along as an extra ones-column in the aligned
    matmuls' stationary operand, landing at an aligned output partition (96).
  - FM heads algebraically reduced: the x^2 @ V^2.T term needs only
    sum_k V_k^2; diff projections are linear combos of the qa/p projections;
    per-head combination is one small stationary matmul per output chunk.
    The two independent elementwise squares run on ACT (idle during FM),
    the product chain on DVE; S-builds run right after the projections so
    all PSUM banks free before the next aligned pass (its matmuls overlap
    the builds), and the combine matmuls+outputs interleave into that pass.
"""
import numpy as np

L_FULL = 2048
D = 600
U = 300
KFM = 5
N_CORES = 8
SCALE = float(1.0 / np.sqrt(np.float32(D)))

DCH = [(0, 128), (128, 128), (256, 128), (384, 128), (512, 88)]   # D chunks
UCH = [(0, 128), (128, 128), (256, 44)]                           # U chunks
ONES_COL = 608        # column in the 640-wide natural tile holding the ones
ONES_ROW = 96         # output partition where the denominator row lands
NATW = 640


def _emit(nc, L):
    import concourse.bass as bass
    import concourse.mybir as mybir
    import concourse.tile as tile
    from concourse.masks import make_identity
    from contextlib import ExitStack

    f32 = mybir.dt.float32
    bf16 = mybir.dt.bfloat16
    fp8 = mybir.dt.float8e4
    AF = mybir.ActivationFunctionType
    ds = bass.ds

    LT = L // 128               # l tiles
    NCW = min(512, L)           # moving-dim chunk width
    NCX = L // NCW              # chunks per L
    TG = 4 if LT % 4 == 0 else 1  # l-tiles per transpose psum batch

    x_d = nc.dram_tensor("x", [2, L, D], bf16, kind="ExternalInput")
    wp_d = nc.dram_tensor("wpack", [10, 128, U], bf16, kind="ExternalInput")
    sp_d = nc.dram_tensor("statp", [10, 128, 36], bf16, kind="ExternalInput")
    c2_d = nc.dram_tensor("comb2", [128, 6], bf16, kind="ExternalInput")
    bp_d = nc.dram_tensor("biasp", [128, 6], f32, kind="ExternalInput")
    w0_d = nc.dram_tensor("w0col", [3, 2], f32, kind="ExternalInput")
    out_d = nc.dram_tensor("out", [2, 3, L], f32, kind="ExternalOutput")

    u16 = mybir.dt.uint16

    with tile.TileContext(nc) as tc, ExitStack() as ctx:
        const = ctx.enter_context(tc.tile_pool(name="const", bufs=1))
        big = ctx.enter_context(tc.tile_pool(name="big", bufs=1))
        epool = ctx.enter_context(tc.tile_pool(name="epool", bufs=LT // 2))
        natp = ctx.enter_context(tc.tile_pool(name="natp", bufs=LT))
        nf32p = ctx.enter_context(tc.tile_pool(name="nf32p", bufs=6))
        fmt = ctx.enter_context(tc.tile_pool(name="fmt", bufs=4))
        sp = ctx.enter_context(tc.tile_pool(name="sp", bufs=2))
        rp = ctx.enter_context(tc.tile_pool(name="rp", bufs=2))
        ob = ctx.enter_context(tc.tile_pool(name="ob", bufs=1))
        ps = ctx.enter_context(tc.tile_pool(name="ps", bufs=8, space="PSUM"))

        def pst(p_cnt=128, w=NCW):
            return ps.tile([p_cnt, w], f32, tag="ps", name="pst")

        # ------- constants (packed loads on the scalar HWDGE queue) -------
        identb = const.tile([128, 128], bf16, tag="identb")
        make_identity(nc, identb)
        onesb = const.tile([128, 128], bf16, tag="onesb")
        nc.vector.memset(onesb[:], 1.0)
        w0sb = const.tile([3, 2], f32, tag="w0sb")
        nc.scalar.dma_start(w0sb[:], w0_d[:])

        # weights / stationaries arrive pre-packed as bf16: straight DMA loads
        Wall = const.tile([128, 10 * U], bf16, tag="Wall")
        nc.scalar.dma_start(
            Wall[:].rearrange("p (t c) -> p t c", t=10),
            wp_d[:].rearrange("t p c -> p t c"))
        Wsb = [[Wall[:, ds((t * 5 + k) * U, U)] for k in range(5)]
               for t in range(2)]

        Sall = const.tile([128, 360], bf16, tag="Sall")
        nc.scalar.dma_start(
            Sall[:].rearrange("p (t c) -> p t c", t=10),
            sp_d[:].rearrange("t p c -> p t c"))
        stat = [[Sall[:, ds((s * 5 + k) * 36, 36)] for k in range(5)]
                for s in range(2)]

        cb2 = const.tile([128, 6], bf16, tag="cb2")
        nc.scalar.dma_start(cb2[:], c2_d[:])

        bsb = const.tile([128, 6], f32, tag="bsb")
        nc.scalar.dma_start(bsb[:], bp_d[:])

        # ---------------- phase 1: transpose inputs -> pT/qT (bf16 [d, L]) ----
        xT = [[], []]
        for t in range(2):
            for k in range(len(DCH)):
                xT[t].append(big.tile([128, L], bf16, tag=f"xT{t}_{k}",
                                      name=f"xT{t}_{k}"))
        # phase 1+2 interleaved per l-group: transpose inputs -> pT/qT, then
        # the dense matmuls for that group's columns (keeps PE fed during the
        # next group's DMA + cast)
        # u-chunks 0,1 live as one fp8 PAIR tile (DoubleRow operand for the
        # affinity matmuls); the 44-row chunk 2 stays bf16 (base-0 + base-64)
        dTP = [big.tile([128, 2, L], fp8, tag=f"dTP{t}", name=f"dTP{t}")
               for t in range(2)]
        dT2 = [big.tile([128, L], fp8, tag=f"dT2{t}", name=f"dT2{t}")
               for t in range(2)]
        # E2 = byte-transposed E1 (exp(A)^T), written by DMA xbar transposes.
        # Layout [r, g, p, s]: q-row 256*g + 2*r + s, p-col p (adjacent-pair
        # DoubleRow convention over q).
        e2all = big.tile([128, LT // 2, L, 2], fp8, tag="e2all", name="e2all")
        # nats[1] (query): standard pairing (a, j) <-> p-row 256*pi + 128*j + a
        # (matches E1's exp-written slot layout, contraction over p).
        # nats[0] (passage): ADJACENT pairing (r, s) <-> q-row 256*pi + 2*r + s
        # (matches e2all, contraction over q), built from a row-interleaved
        # second load of the passage tensor.
        nats = [[None] * (LT // 2) for _ in range(2)]
        x0i = x_d[0].rearrange("(g r s) d -> g r s d", r=128, s=2)

        DRm = mybir.MatmulPerfMode.DoubleRow
        E1 = [None] * (LT // 2)

        def aff_chunk(pi2, nx):
            """Affinity rows 256*pi2..+256 x cols nx*NCW..+NCW -> exp -> E1."""
            if E1[pi2] is None:
                E1[pi2] = epool.tile([128, 2, L], fp8, tag="E",
                                     name=f"E1_{pi2}")
            e = E1[pi2]
            nsl = ds(nx * NCW, NCW)
            accs = (pst(), pst())
            for j in (0, 1):
                isl = ds((2 * pi2 + j) * 128, 128)
                # u-chunks 0+1 in one fp8 DoubleRow pass
                nc.tensor.matmul(accs[j][:, :], dTP[0][:, :, isl],
                                 dTP[1][:, :, nsl],
                                 start=True, stop=False, perf_mode=DRm)
            # 44-row K chunk: the pair's two matmuls go to disjoint PE
            # row-groups and run concurrently
            nc.tensor.matmul(accs[0][:, :],
                             dT2[0][0:44, ds(2 * pi2 * 128, 128)],
                             dT2[1][0:44, nsl],
                             start=False, stop=True, tile_position=(0, 0))
            nc.tensor.matmul(accs[1][:, :],
                             dT2[0][64:108, ds((2 * pi2 + 1) * 128, 128)],
                             dT2[1][64:108, nsl],
                             start=False, stop=True, tile_position=(64, 0))
            for j in (0, 1):
                nc.scalar.activation(e[:, j, nsl], accs[j][:, :],
                                     AF.Exp, scale=SCALE)

        for g in range(LT // TG):
            gw = TG * 128
            for pi in range(g * TG // 2, (g + 1) * TG // 2):
                nf2 = nf32p.tile([128, 2, D], bf16, tag="nf2", name="nf2",
                                 bufs=1)
                nc.sync.dma_start(nf2[:], x0i[pi])
                nt0 = natp.tile([128, 2, NATW], fp8, tag="nat",
                                name=f"nat0_{pi}")
                nats[0][pi] = nt0
                nc.gpsimd.memset(nt0[:, :, D:NATW], 0.0)
                nc.gpsimd.memset(nt0[:, :, ONES_COL:ONES_COL + 1], 1.0)
                if pi % 2 == 0:
                    nc.vector.tensor_copy(nt0[:, :, 0:D], nf2[:])
                else:
                    nc.gpsimd.tensor_copy(nt0[:, :, 0:D], nf2[:])
            for t in range(2):
                # 2 d-chunks per bf16 psum tile (same 2KB bank footprint as
                # one f32 slot) -> 3 slots instead of 5, more slot headroom
                # for the dense accumulators and the next group's transposes
                pjs2 = [ps.tile([128, 2 * NCW], bf16, tag="ps", name="pjs")
                        for _ in range((len(DCH) + 1) // 2)]
                pjs = [pjs2[k // 2][:, ds((k % 2) * NCW, NCW)]
                       for k in range(len(DCH))]
                for ii in range(TG):
                    i = g * TG + ii
                    nf = nf32p.tile([128, D], bf16, tag="nf", name="nf",
                                    bufs=4)
                    eng = nc.sync if (g == 0 or i % 2 == 0) else nc.scalar
                    eng.dma_start(nf[:], x_d[t, ds(i * 128, 128), :])
                    nfb = nf
                    # build the fp8 natural-layout pair tile (DoubleRow operand
                    # of the aligned matmuls) from the same load; only t=1 --
                    # t=0 is built adjacent-paired from the nf2 loads above
                    pi, j = i // 2, i % 2
                    if t == 1:
                        if j == 0:
                            nats[1][pi] = natp.tile([128, 2, NATW], fp8,
                                                    tag="nat",
                                                    name=f"nat1_{pi}")
                            nc.gpsimd.memset(nats[1][pi][:, :, D:NATW], 0.0)
                            nc.gpsimd.memset(
                                nats[1][pi][:, :, ONES_COL:ONES_COL + 1], 1.0)
                        nt = nats[1][pi]
                        # DVE/Pool: ACT is the bottleneck of this merged phase
                        if j == 0:
                            nc.vector.tensor_copy(nt[:, j, 0:D], nf[:])
                        else:
                            nc.gpsimd.tensor_copy(nt[:, j, 0:D], nf[:])
                    for k, (doff, dcnt) in enumerate(DCH):
                        nc.tensor.transpose(
                            pjs[k][:dcnt, ds(ii * 128, 128)],
                            nfb[:, ds(doff, dcnt)], identb[:])
                for k, (doff, dcnt) in enumerate(DCH):
                    # all on DVE: ACT is the bottleneck of this merged phase
                    nc.vector.tensor_copy(xT[t][k][:dcnt, ds(g * gw, gw)],
                                          pjs[k][:dcnt, ds(0, gw)])
            if gw == NCW:
                for t in range(2):
                    for m, (uoff, ucnt) in enumerate(UCH[:2]):
                        acc = pst()
                        for k, (doff, dcnt) in enumerate(DCH):
                            nc.tensor.matmul(
                                acc[:ucnt, :],
                                Wsb[t][k][:dcnt, ds(uoff, ucnt)],
                                xT[t][k][:dcnt, ds(g * NCW, NCW)],
                                start=(k == 0), stop=(k == len(DCH) - 1))
                        if g >= 2:
                            # late groups: ACT is saturated by the affinity
                            # exp wavefront; fused add+max on the idler DVE
                            nc.vector.tensor_scalar(
                                dTP[t][:, m, ds(g * NCW, NCW)], acc[:ucnt, :],
                                bsb[:ucnt, t * 3 + m: t * 3 + m + 1], 0.0,
                                mybir.AluOpType.add, mybir.AluOpType.max)
                        else:
                            nc.scalar.activation(
                                dTP[t][:, m, ds(g * NCW, NCW)], acc[:ucnt, :],
                                AF.Relu,
                                bias=bsb[:ucnt, t * 3 + m: t * 3 + m + 1])
                # the 44-row M-chunk: both tensors' matmuls in concurrent
                # col-groups (0 and 64) of one psum tile
                uoff, ucnt = UCH[2]
                acc2 = pst()
                for k, (doff, dcnt) in enumerate(DCH):
                    fl = (k == 0, k == len(DCH) - 1)
                    nc.tensor.matmul(
                        acc2[0:ucnt, :],
                        Wsb[0][k][:dcnt, ds(uoff, ucnt)],
                        xT[0][k][:dcnt, ds(g * NCW, NCW)],
                        start=fl[0], stop=fl[1], tile_position=(0, 0),
                        skip_group_check=True)
                    nc.tensor.matmul(
                        acc2[64:64 + ucnt, :],
                        Wsb[1][k][:dcnt, ds(uoff, ucnt)],
                        xT[1][k][:dcnt, ds(g * NCW, NCW)],
                        start=fl[0], stop=fl[1], tile_position=(0, 64),
                        skip_group_check=True)
                for t in range(2):
                    pb = t * 64
                    for dst in (0, 64):
                        # evict to base 0 (affinity k2 slice) and base 64
                        # (its row-pair partner slice)
                        if g >= 2:
                            nc.vector.tensor_scalar(
                                dT2[t][dst:dst + ucnt, ds(g * NCW, NCW)],
                                acc2[pb:pb + ucnt, :],
                                bsb[:ucnt, t * 3 + 2: t * 3 + 3], 0.0,
                                mybir.AluOpType.add, mybir.AluOpType.max)
                        else:
                            nc.scalar.activation(
                                dT2[t][dst:dst + ucnt, ds(g * NCW, NCW)],
                                acc2[pb:pb + ucnt, :], AF.Relu,
                                bias=bsb[:ucnt, t * 3 + 2: t * 3 + 3])
                # wavefront affinity: every (row-pair, col-chunk) whose dense
                # inputs just became ready -- spreads the exp (ACT) load into
                # the DMA-paced transpose/dense phase
                for pi2 in range(2 * g, 2 * g + 2):
                    for nx in range(g + 1):
                        aff_chunk(pi2, nx)
                for pi2 in range(0, 2 * g):
                    aff_chunk(pi2, g)
        if TG * 128 != NCW:
            for t in range(2):
                for m, (uoff, ucnt) in enumerate(UCH):
                    for nx in range(NCX):
                        acc = pst()
                        for k, (doff, dcnt) in enumerate(DCH):
                            nc.tensor.matmul(
                                acc[:ucnt, :],
                                Wsb[t][k][:dcnt, ds(uoff, ucnt)],
                                xT[t][k][:dcnt, ds(nx * NCW, NCW)],
                                start=(k == 0), stop=(k == len(DCH) - 1))
                        if m < 2:
                            nc.scalar.activation(
                                dTP[t][:, m, ds(nx * NCW, NCW)], acc[:ucnt, :],
                                AF.Relu,
                                bias=bsb[:ucnt, t * 3 + m: t * 3 + m + 1])
                        else:
                            for dst in (0, 64):
                                nc.scalar.activation(
                                    dT2[t][dst:dst + ucnt, ds(nx * NCW, NCW)],
                                    acc[:ucnt, :], AF.Relu,
                                    bias=bsb[:ucnt, t * 3 + m: t * 3 + m + 1])

        # E2: as each E1 pair tile completes, two xbar DMA transposes copy it
        # (viewed as uint16 q-pairs) into e2all, building exp(A)^T on the idle
        # DMA engines instead of a second affinity+exp pass.
        for pi in range(LT // 2):
            for j in range(2):
                inap = E1[pi][:, j, :].bitcast(u16).rearrange(
                    "p (g u) -> p g u", g=LT // 2)
                pc = (2 * pi + j) * 128
                outap = e2all[:, :, ds(pc, 128), :].bitcast(u16).squeeze()
                nc.sync.dma_start_transpose(outap, inap)

        # helpers ------------------------------------------------------------
        def aligned_T(nats, mov, side_tag, hooks=()):
            """alT tiles [d,L] bf16 = normalized aligned.T, via ones-row trick.

            mov(pi, nsl) yields the fp8 DoubleRow moving AP [128, 2, |nsl|];
            the stationary nats[pi] must pair contraction rows the same way.
            """
            alT = [big.tile([128, L], bf16, tag=f"alT{k}", name=f"alT{side_tag}{k}")
                   for k in range(len(DCH))]
            R = big.tile([128, L], bf16, tag="R", name=f"R{side_tag}")
            NP = LT // 2
            DR = mybir.MatmulPerfMode.DoubleRow
            hooks = list(hooks)
            if hooks:
                hooks.pop(0)()
            # pass A: last d-chunk (88 rows) + ones row at partition 96
            ps4 = [pst() for _ in range(NCX)]
            for pi in range(NP):
                for nx in range(NCX):
                    nc.tensor.matmul(ps4[nx][:, :],
                                     nats[pi][:, :, ds(512, 128)],
                                     mov(pi, ds(nx * NCW, NCW)),
                                     start=(pi == 0), stop=(pi == NP - 1),
                                     perf_mode=DR)
            for nx in range(NCX):
                rrb = rp.tile([128, NCW], bf16, tag="rrb", name="rrb")
                with nc.allow_low_precision(reason="R is consumed as bf16"):
                    nc.vector.reciprocal(rrb[ONES_ROW:ONES_ROW + 1, :],
                                         ps4[nx][ONES_ROW:ONES_ROW + 1, :])
                bc = pst()
                nc.tensor.matmul(bc[:, :], onesb[ONES_ROW:ONES_ROW + 1, 0:128],
                                 rrb[ONES_ROW:ONES_ROW + 1, :],
                                 start=True, stop=True,
                                 tile_position=(ONES_ROW, 0))
                nc.scalar.copy(R[:, ds(nx * NCW, NCW)], bc[:, :])
                nc.vector.tensor_mul(alT[4][0:88, ds(nx * NCW, NCW)],
                                     ps4[nx][0:88, :],
                                     R[0:88, ds(nx * NCW, NCW)])
            # passes B, C: d-chunks 0..3, two at a time
            for mm0 in (0, 2):
                if hooks:
                    hooks.pop(0)()
                accs = {}
                for m in (mm0, mm0 + 1):
                    for nx in range(NCX):
                        accs[(m, nx)] = pst()
                for pi in range(NP):
                    for m in (mm0, mm0 + 1):
                        for nx in range(NCX):
                            nc.tensor.matmul(accs[(m, nx)][:, :],
                                             nats[pi][:, :, ds(m * 128, 128)],
                                             mov(pi, ds(nx * NCW, NCW)),
                                             start=(pi == 0),
                                             stop=(pi == NP - 1),
                                             perf_mode=DR)
                for m in (mm0, mm0 + 1):
                    for nx in range(NCX):
                        nc.vector.tensor_mul(alT[m][:, ds(nx * NCW, NCW)],
                                             accs[(m, nx)][:, :],
                                             R[:, ds(nx * NCW, NCW)])
            while hooks:
                hooks.pop(0)()
            return alT, R

        def fm_proj(s, xTs, bTs):
            """FM projection matmuls for one side; returns live PSUM groups.

            d-chunk-outer loop: the elementwise temps are built full-width once
            per chunk (fewer DVE ops, deeper PE overlap); all four N-chunks'
            projection groups accumulate simultaneously (8 PSUM banks).
            """
            P1s = [ps.tile([128, NCW], f32, tag="ps", name="P1")
                   for _ in range(NCX)]
            P2s = [ps.tile([128, NCW], f32, tag="ps", name="P2")
                   for _ in range(NCX)]
            nk = len(DCH)
            for k, (doff, dcnt) in enumerate(DCH):
                x_fl = xTs[k][:dcnt, :]
                b_fl = bTs[k][:dcnt, :]
                tx2 = fmt.tile([128, L], bf16, tag="fmt", name="tx2")
                tb2 = fmt.tile([128, L], bf16, tag="fmt", name="tb2")
                txm = fmt.tile([128, L], bf16, tag="fmt", name="txm")
                txm2 = fmt.tile([128, L], bf16, tag="fmt", name="txm2")
                # the two independent squares go to ACT (idle through the FM
                # phases); the txm -> txm2 chain stays on the faster DVE
                nc.scalar.activation(tx2[:dcnt, :], x_fl, AF.Square)
                nc.scalar.activation(tb2[:dcnt, :], b_fl, AF.Square)
                nc.vector.tensor_mul(txm[:dcnt, :], x_fl, b_fl)
                nc.vector.tensor_mul(txm2[:dcnt, :], txm[:dcnt, :],
                                     txm[:dcnt, :])
                st = stat[s][k]
                fl = (k == 0, k == nk - 1)
                for nx in range(NCX):
                    nsl = ds(nx * NCW, NCW)
                    P1, P2 = P1s[nx], P2s[nx]
                    nc.tensor.matmul(P1[0:12, :], st[:dcnt, 0:12],
                                     xTs[k][:dcnt, nsl],
                                     start=fl[0], stop=fl[1],
                                     tile_position=(0, 0),
                                     skip_group_check=True)
                    nc.tensor.matmul(P1[32:44, :], st[:dcnt, 12:24],
                                     bTs[k][:dcnt, nsl],
                                     start=fl[0], stop=fl[1],
                                     tile_position=(0, 32),
                                     skip_group_check=True)
                    nc.tensor.matmul(P1[64:65, :], st[:dcnt, 35:36],
                                     txm2[:dcnt, nsl], start=fl[0], stop=fl[1],
                                     tile_position=(0, 64),
                                     skip_group_check=True)
                    nc.tensor.matmul(P2[0:2, :], st[:dcnt, 24:26],
                                     tx2[:dcnt, nsl], start=fl[0], stop=fl[1],
                                     tile_position=(0, 0),
                                     skip_group_check=True)
                    nc.tensor.matmul(P2[32:34, :], st[:dcnt, 26:28],
                                     tb2[:dcnt, nsl], start=fl[0], stop=fl[1],
                                     tile_position=(0, 32),
                                     skip_group_check=True)
                    nc.tensor.matmul(P2[64:71, :], st[:dcnt, 28:35],
                                     txm[:dcnt, nsl], start=fl[0], stop=fl[1],
                                     tile_position=(0, 64),
                                     skip_group_check=True)
            return P1s, P2s

        def fm_comb_build(P1s, P2s, nx, R):
            """S-build (ACT/DVE only) for one N-chunk: evict + square the FM
            groups into S1/S2. Frees the psum banks early so the next
            aligned_T pass's matmuls overlap with the builds."""
            P1, P2 = P1s[nx], P2s[nx]
            S1 = sp.tile([128, NCW], bf16, tag="S1", name="S1", bufs=4)
            S2 = sp.tile([128, NCW], bf16, tag="S2", name="S2", bufs=4)
            nc.gpsimd.memset(S1[:], 0.0)
            nc.gpsimd.memset(S2[:], 0.0)
            # split evictions ACT/DVE so the S-build runs in parallel
            nc.scalar.copy(S1[0:12, :], P1[0:12, :])
            nc.scalar.copy(S1[32:44, :], P1[32:44, :])
            nc.vector.tensor_copy(S1[64:66, :], P2[0:2, :])
            nc.vector.tensor_copy(S1[96:98, :], P2[32:34, :])
            nc.vector.tensor_copy(S2[0:7, :], P2[64:71, :])
            nc.vector.tensor_copy(S2[32:33, :], P1[64:65, :])
            # B-group Vd columns carry -Vd, so diff quads are also an add.
            TA = sp.tile([10, NCW], f32, tag="TA", name="TA", bufs=2)
            nc.vector.tensor_add(TA[0:10, :], P1[0:10, :], S1[32:42, :])
            nc.scalar.activation(S2[64:74, :], TA[:, :], AF.Square)
            nc.scalar.activation(S2[96:101, :], S2[0:5, :], AF.Square)
            return S1, S2

        def fm_comb_mm(s, Ss, nx):
            """Combine matmuls + bias + output DMA for one N-chunk."""
            S1, S2 = Ss[nx]
            nsl = ds(nx * NCW, NCW)
            cps = ps.tile([3, NCW], f32, tag="ps", name="cps")
            nc.tensor.matmul(cps[:, :], cb2[0:98, 0:3], S1[0:98, :],
                             start=True, stop=False)
            nc.tensor.matmul(cps[:, :], cb2[0:101, 3:6], S2[0:101, :],
                             start=False, stop=True)
            o = ob.tile([3, NCW], f32, tag="ob", name="o")
            nc.scalar.activation(o[:, :], cps[:, :], AF.Identity,
                                 bias=w0sb[:, s:s + 1])
            nc.sync.dma_start(out_d[s, :, nsl], o[:, :])

        # ---------------- main flow ----------------
        qaT, Rq = aligned_T(nats[1],
                            lambda pi, nsl: E1[pi][:, :, nsl], "q")
        P1s, P2s = fm_proj(0, qaT, xT[0])         # passage-side projections
        # S-builds (ACT/DVE) free all 8 psum banks up front; the combine
        # matmuls + outputs interleave with paT's passes so PE never stalls
        Ss0 = [fm_comb_build(P1s, P2s, nx, Rq) for nx in range(NCX)]
        # passage_aligned.T from the DMA-transposed exp(A)^T (adjacent q-pairs)
        paT, Rp = aligned_T(nats[0],
                            lambda pi, nsl: e2all[:, pi, nsl, :].rearrange(
                                "p n s -> p s n"), "p",
                            hooks=[lambda nx=nx: fm_comb_mm(0, Ss0, nx)
                                   for nx in range(NCX)])
        P1s1, P2s1 = fm_proj(1, paT, xT[1])       # query-side projections
        # tail: interleave build/mm per N-chunk so each combine's matmuls
        # start as soon as its own S-build is done (nothing follows to
        # overlap with, so serialization here is pure wall time)
        Ss1 = []
        for nx in range(NCX):
            Ss1.append(fm_comb_build(P1s1, P2s1, nx, Rp))
            fm_comb_mm(1, Ss1, nx)


def _host_prep(W1, b1, W2, b2, cat_w0, cat_w, cat_V, dm_w0, dm_w, dm_V):
    stat = np.zeros((2, D, 36), np.float32)
    for s in range(2):
        ci, di, mi = s, s, s + 2
        Va = cat_V[ci][:, :D]
        Vb = cat_V[ci][:, D:]
        Vd = dm_V[di]
        Vm = dm_V[mi]
        stat[s, :, 0:5] = Va.T
        stat[s, :, 5:10] = Vd.T
        stat[s, :, 10] = cat_w[ci, :D]
        stat[s, :, 11] = dm_w[di]
        stat[s, :, 12:17] = Vb.T
        stat[s, :, 17:22] = -Vd.T   # negated: quad build is then a single add
        stat[s, :, 22] = cat_w[ci, D:]
        stat[s, :, 23] = dm_w[di]
        stat[s, :, 24] = (Va ** 2).sum(0)
        stat[s, :, 25] = (Vd ** 2).sum(0)
        stat[s, :, 26] = (Vb ** 2).sum(0)
        stat[s, :, 27] = (Vd ** 2).sum(0)
        stat[s, :, 28:33] = Vm.T
        stat[s, :, 33] = dm_w[mi]
        stat[s, :, 34] = (Vd ** 2).sum(0)
        stat[s, :, 35] = (Vm ** 2).sum(0)

    # packed combine matrices: S1 = [X@0, B@32, X2@64, B2@96],
    # S2 = [M@0, M2@32, TQ@64, TQM@96]
    comb2 = np.zeros((128, 6), np.float32)
    C1, C2 = comb2[:, 0:3], comb2[:, 3:6]
    C1[10, 0] = 1.0     # x@w_cat -> c_cat
    C1[11, 1] = 1.0     # x@w_d -> c_diff
    C1[32 + 10, 0] = 1.0
    C1[32 + 11, 1] = -1.0
    C1[64, 0] = -0.5    # x2@u_cat
    C1[65, 1] = -0.5    # x2@u_d
    C1[96, 0] = -0.5    # b2@u_cat
    C1[97, 1] = -0.5    # b2@u_d
    C2[5, 2] = 1.0      # mul@w_m
    C2[6, 1] = 1.0      # mul@u_d (from -0.5 * -2)
    C2[32, 2] = -0.5    # mul2@u_m
    C2[64:69, 0] = 0.5  # cat quads
    C2[69:74, 1] = 0.5  # diff quads
    C2[96:101, 2] = 0.5  # mul quads

    # packed per-d-chunk weights / stationaries; the dense bias rides as an
    # extra stationary row (matched by a ones row in xT's last chunk)
    wpack = np.zeros((10, 128, U), np.float32)
    statp = np.zeros((10, 128, 36), np.float32)
    for t, W in enumerate((W1, W2)):
        for k, (doff, dcnt) in enumerate(DCH):
            wpack[t * 5 + k, :dcnt] = W[doff:doff + dcnt]
    for s in range(2):
        for k, (doff, dcnt) in enumerate(DCH):
            statp[s * 5 + k, :dcnt] = stat[s, doff:doff + dcnt]

    biasp = np.zeros((128, 6), np.float32)
    for t, b in enumerate((b1, b2)):
        for m, (uoff, ucnt) in enumerate(UCH):
            biasp[:ucnt, t * 3 + m] = b[uoff:uoff + ucnt]

    w0col = np.zeros((3, 2), np.float32)
    for s in range(2):
        w0col[0, s] = cat_w0[s, 0]
        w0col[1, s] = dm_w0[s, 0]
        w0col[2, s] = dm_w0[s + 2, 0]
    return wpack, statp, comb2, biasp, w0col


_PROG = None


def _get_prog():
    global _PROG
    if _PROG is None:
        from concourse import bacc
        nc = bacc.Bacc(None, target_bir_lowering=False)
        _emit(nc, L_FULL)
        nc.finalize()
        _PROG = nc
    return _PROG


def _in_maps(stack_input, W1, b1, W2, b2, fm_cat_w0, fm_cat_w, fm_cat_V,
             fm_dm_w0, fm_dm_w, fm_dm_V):
    import ml_dtypes
    f = lambda a: np.ascontiguousarray(np.asarray(a, np.float32))
    bf = lambda a: np.ascontiguousarray(np.asarray(a, ml_dtypes.bfloat16))
    stack_input = bf(stack_input)
    wpack, statp, comb2, biasp, w0col = _host_prep(
        f(W1), f(b1), f(W2), f(b2), f(fm_cat_w0), f(fm_cat_w), f(fm_cat_V),
        f(fm_dm_w0), f(fm_dm_w), f(fm_dm_V))
    common = {"wpack": bf(wpack), "statp": bf(statp), "comb2": bf(comb2),
              "biasp": biasp, "w0col": w0col}
    return [dict(common, x=np.ascontiguousarray(stack_input[:, b]))
            for b in range(N_CORES)]


def kernel(stack_input, W1, b1, W2, b2, fm_cat_w0, fm_cat_w, fm_cat_V,
           fm_dm_w0, fm_dm_w, fm_dm_V):
    from concourse.bass_utils import run_bass_kernel_spmd

    in_maps = _in_maps(stack_input, W1, b1, W2, b2, fm_cat_w0, fm_cat_w,
                       fm_cat_V, fm_dm_w0, fm_dm_w, fm_dm_V)
    nc = _get_prog()
    res = run_bass_kernel_spmd(nc, in_maps, core_ids=list(range(N_CORES)))
    outs = [r["out"] for r in res.results]            # each [2, 3, L]
    fp = np.stack([o[0].T for o in outs]).astype(np.float32)   # [8, L, 3]
    fq = np.stack([o[1].T for o in outs]).astype(np.float32)
    return fp, fq



# revision 71
# speedup vs baseline: 1.0026x; 1.0010x over previous
"""Trainium2 Bass kernel for nn_BAC_15152644620305.

Per batch element (1 per NeuronCore, 8 cores):
  p_dense = relu(p @ W1 + b1); q_dense = relu(q @ W2 + b2)
  A = (p_dense @ q_dense.T) / sqrt(600)
  passage_aligned = softmax_rows(A) @ passage ; query_aligned = softmax_cols(A).T @ query
  6 factorization-machine heads on {concat, diff, mul} pairs -> [L, 3] x 2 outputs.

Implementation notes:
  - All heavy matmuls in bf16 (1 cyc/row on PE), fp32 PSUM accumulation;
    aligned/affinity contractions use fp8e4m3 DoubleRow (0.5 cyc/row).
  - Inputs arrive pre-cast to bf16 (halves input DMA, no on-chip f32 casts);
    dense weights/stationaries pre-packed bf16 on the host.
  - Affinity computed ONCE; exp(A) (fp8) is transposed into exp(A)^T by xbar
    DMA transposes (uint16 views batched 8 blocks/instruction) on the
    otherwise-idle DMA engines -- no second affinity+exp pass.  The paT
    contraction consumes the byte-transposed layout via adjacent-(q,q+1)
    DoubleRow pairs; its stationary natural tiles are built adjacent-paired
    directly by a row-interleaved second DMA load of the passage tensor.
  - Affinity is emitted as a wavefront inside the transpose/dense phase:
    each (row-pair, col-chunk) fires as soon as its dense outputs exist,
    spreading the exp (ACT) load into the DMA-paced start.
  - exp without max-subtraction (affinity values are in [0.1, 1.1]).
  - Softmax denominators ride along as an extra ones-column in the aligned
    matmuls' stationary operand, landing at an aligned output partition (96).
  - FM heads algebraically reduced: the x^2 @ V^2.T term needs only
    sum_k V_k^2; diff projections are linear combos of the qa/p projections;
    per-head combination is one small stationary matmul per output chunk.
    The two independent elementwise squares run on ACT (idle during FM),
    the product chain on DVE; S-builds run right after the projections so
    all PSUM banks free before the next aligned pass (its matmuls overlap
    the builds), and the combine matmuls+outputs interleave into that pass.
"""
import numpy as np

L_FULL = 2048
D = 600
U = 300
KFM = 5
N_CORES = 8
SCALE = float(1.0 / np.sqrt(np.float32(D)))

DCH = [(0, 128), (128, 128), (256, 128), (384, 128), (512, 88)]   # D chunks
UCH = [(0, 128), (128, 128), (256, 44)]                           # U chunks
ONES_COL = 608        # column in the 640-wide natural tile holding the ones
ONES_ROW = 96         # output partition where the denominator row lands
NATW = 640


def _emit(nc, L):
    import concourse.bass as bass
    import concourse.mybir as mybir
    import concourse.tile as tile
    from concourse.masks import make_identity
    from contextlib import ExitStack

    f32 = mybir.dt.float32
    bf16 = mybir.dt.bfloat16
    fp8 = mybir.dt.float8e4
    AF = mybir.ActivationFunctionType
    ds = bass.ds

    LT = L // 128               # l tiles
    NCW = min(512, L)           # moving-dim chunk width
    NCX = L // NCW              # chunks per L
    TG = 4 if LT % 4 == 0 else 1  # l-tiles per transpose psum batch

    x_d = nc.dram_tensor("x", [2, L, D], bf16, kind="ExternalInput")
    wp_d = nc.dram_tensor("wpack", [10, 128, U], bf16, kind="ExternalInput")
    sp_d = nc.dram_tensor("statp", [10, 128, 36], bf16, kind="ExternalInput")
    c2_d = nc.dram_tensor("comb2", [128, 6], bf16, kind="ExternalInput")
    bp_d = nc.dram_tensor("biasp", [128, 6], f32, kind="ExternalInput")
    w0_d = nc.dram_tensor("w0col", [3, 2], f32, kind="ExternalInput")
    out_d = nc.dram_tensor("out", [2, 3, L], f32, kind="ExternalOutput")

    u16 = mybir.dt.uint16

    with tile.TileContext(nc) as tc, ExitStack() as ctx:
        const = ctx.enter_context(tc.tile_pool(name="const", bufs=1))
        big = ctx.enter_context(tc.tile_pool(name="big", bufs=1))
        epool = ctx.enter_context(tc.tile_pool(name="epool", bufs=LT // 2))
        natp = ctx.enter_context(tc.tile_pool(name="natp", bufs=LT))
        nf32p = ctx.enter_context(tc.tile_pool(name="nf32p", bufs=6))
        fmt = ctx.enter_context(tc.tile_pool(name="fmt", bufs=4))
        sp = ctx.enter_context(tc.tile_pool(name="sp", bufs=2))
        rp = ctx.enter_context(tc.tile_pool(name="rp", bufs=2))
        ob = ctx.enter_context(tc.tile_pool(name="ob", bufs=1))
        ps = ctx.enter_context(tc.tile_pool(name="ps", bufs=8, space="PSUM"))

        def pst(p_cnt=128, w=NCW):
            return ps.tile([p_cnt, w], f32, tag="ps", name="pst")

        # ------- constants (packed loads on the scalar HWDGE queue) -------
        identb = const.tile([128, 128], bf16, tag="identb")
        make_identity(nc, identb)
        onesb = const.tile([128, 128], bf16, tag="onesb")
        nc.vector.memset(onesb[:], 1.0)
        w0sb = const.tile([3, 2], f32, tag="w0sb")
        nc.scalar.dma_start(w0sb[:], w0_d[:])

        # weights / stationaries arrive pre-packed as bf16: straight DMA loads
        Wall = const.tile([128, 10 * U], bf16, tag="Wall")
        nc.scalar.dma_start(
            Wall[:].rearrange("p (t c) -> p t c", t=10),
            wp_d[:].rearrange("t p c -> p t c"))
        Wsb = [[Wall[:, ds((t * 5 + k) * U, U)] for k in range(5)]
               for t in range(2)]

        Sall = const.tile([128, 360], bf16, tag="Sall")
        nc.scalar.dma_start(
            Sall[:].rearrange("p (t c) -> p t c", t=10),
            sp_d[:].rearrange("t p c -> p t c"))
        stat = [[Sall[:, ds((s * 5 + k) * 36, 36)] for k in range(5)]
                for s in range(2)]

        cb2 = const.tile([128, 6], bf16, tag="cb2")
        nc.scalar.dma_start(cb2[:], c2_d[:])

        bsb = const.tile([128, 6], f32, tag="bsb")
        nc.scalar.dma_start(bsb[:], bp_d[:])

        # ---------------- phase 1: transpose inputs -> pT/qT (bf16 [d, L]) ----
        xT = [[], []]
        for t in range(2):
            for k in range(len(DCH)):
                xT[t].append(big.tile([128, L], bf16, tag=f"xT{t}_{k}",
                                      name=f"xT{t}_{k}"))
        # phase 1+2 interleaved per l-group: transpose inputs -> pT/qT, then
        # the dense matmuls for that group's columns (keeps PE fed during the
        # next group's DMA + cast)
        # u-chunks 0,1 live as one fp8 PAIR tile (DoubleRow operand for the
        # affinity matmuls); the 44-row chunk 2 stays bf16 (base-0 + base-64)
        dTP = [big.tile([128, 2, L], fp8, tag=f"dTP{t}", name=f"dTP{t}")
               for t in range(2)]
        dT2 = [big.tile([128, L], fp8, tag=f"dT2{t}", name=f"dT2{t}")
               for t in range(2)]
        # E2 = byte-transposed E1 (exp(A)^T), written by DMA xbar transposes.
        # Layout [r, g, p, s]: q-row 256*g + 2*r + s, p-col p (adjacent-pair
        # DoubleRow convention over q).
        e2all = big.tile([128, LT // 2, L, 2], fp8, tag="e2all", name="e2all")
        # nats[1] (query): standard pairing (a, j) <-> p-row 256*pi + 128*j + a
        # (matches E1's exp-written slot layout, contraction over p).
        # nats[0] (passage): ADJACENT pairing (r, s) <-> q-row 256*pi + 2*r + s
        # (matches e2all, contraction over q), built from a row-interleaved
        # second load of the passage tensor.
        nats = [[None] * (LT // 2) for _ in range(2)]
        x0i = x_d[0].rearrange("(g r s) d -> g r s d", r=128, s=2)

        DRm = mybir.MatmulPerfMode.DoubleRow
        E1 = [None] * (LT // 2)

        def aff_chunk(pi2, nx):
            """Affinity rows 256*pi2..+256 x cols nx*NCW..+NCW -> exp -> E1."""
            if E1[pi2] is None:
                E1[pi2] = epool.tile([128, 2, L], fp8, tag="E",
                                     name=f"E1_{pi2}")
            e = E1[pi2]
            nsl = ds(nx * NCW, NCW)
            accs = (pst(), pst())
            for j in (0, 1):
                isl = ds((2 * pi2 + j) * 128, 128)
                # u-chunks 0+1 in one fp8 DoubleRow pass
                nc.tensor.matmul(accs[j][:, :], dTP[0][:, :, isl],
                                 dTP[1][:, :, nsl],
                                 start=True, stop=False, perf_mode=DRm)
            # 44-row K chunk: the pair's two matmuls go to disjoint PE
            # row-groups and run concurrently
            nc.tensor.matmul(accs[0][:, :],
                             dT2[0][0:44, ds(2 * pi2 * 128, 128)],
                             dT2[1][0:44, nsl],
                             start=False, stop=True, tile_position=(0, 0))
            nc.tensor.matmul(accs[1][:, :],
                             dT2[0][64:108, ds((2 * pi2 + 1) * 128, 128)],
                             dT2[1][64:108, nsl],
                             start=False, stop=True, tile_position=(64, 0))
            for j in (0, 1):
                nc.scalar.activation(e[:, j, nsl], accs[j][:, :],
                                     AF.Exp, scale=SCALE)

        for g in range(LT // TG):
            gw = TG * 128
            for pi in range(g * TG // 2, (g + 1) * TG // 2):
                nf2 = nf32p.tile([128, 2, D], bf16, tag="nf2", name="nf2",
                                 bufs=1)
                nc.sync.dma_start(nf2[:], x0i[pi])
                nt0 = natp.tile([128, 2, NATW], fp8, tag="nat",
                                name=f"nat0_{pi}")
                nats[0][pi] = nt0
                nc.gpsimd.memset(nt0[:, :, D:NATW], 0.0)
                nc.gpsimd.memset(nt0[:, :, ONES_COL:ONES_COL + 1], 1.0)
                if pi % 2 == 0:
                    nc.vector.tensor_copy(nt0[:, :, 0:D], nf2[:])
                else:
                    nc.gpsimd.tensor_copy(nt0[:, :, 0:D], nf2[:])
            for t in range(2):
                # 2 d-chunks per bf16 psum tile (same 2KB bank footprint as
                # one f32 slot) -> 3 slots instead of 5, more slot headroom
                # for the dense accumulators and the next group's transposes
                pjs2 = [ps.tile([128, 2 * NCW], bf16, tag="ps", name="pjs")
                        for _ in range((len(DCH) + 1) // 2)]
                pjs = [pjs2[k // 2][:, ds((k % 2) * NCW, NCW)]
                       for k in range(len(DCH))]
                for ii in range(TG):
                    i = g * TG + ii
                    nf = nf32p.tile([128, D], bf16, tag="nf", name="nf",
                                    bufs=4)
                    eng = nc.sync if (g == 0 or i % 2 == 0) else nc.scalar
                    eng.dma_start(nf[:], x_d[t, ds(i * 128, 128), :])
                    nfb = nf
                    # build the fp8 natural-layout pair tile (DoubleRow operand
                    # of the aligned matmuls) from the same load; only t=1 --
                    # t=0 is built adjacent-paired from the nf2 loads above
                    pi, j = i // 2, i % 2
                    if t == 1:
                        if j == 0:
                            nats[1][pi] = natp.tile([128, 2, NATW], fp8,
                                                    tag="nat",
                                                    name=f"nat1_{pi}")
                            nc.gpsimd.memset(nats[1][pi][:, :, D:NATW], 0.0)
                            nc.gpsimd.memset(
                                nats[1][pi][:, :, ONES_COL:ONES_COL + 1], 1.0)
                        nt = nats[1][pi]
                        # DVE/Pool: ACT is the bottleneck of this merged phase
                        if j == 0:
                            nc.vector.tensor_copy(nt[:, j, 0:D], nf[:])
                        else:
                            nc.gpsimd.tensor_copy(nt[:, j, 0:D], nf[:])
                    for k, (doff, dcnt) in enumerate(DCH):
                        nc.tensor.transpose(
                            pjs[k][:dcnt, ds(ii * 128, 128)],
                            nfb[:, ds(doff, dcnt)], identb[:])
                for k, (doff, dcnt) in enumerate(DCH):
                    # all on DVE: ACT is the bottleneck of this merged phase
                    nc.vector.tensor_copy(xT[t][k][:dcnt, ds(g * gw, gw)],
                                          pjs[k][:dcnt, ds(0, gw)])
            if gw == NCW:
                for t in range(2):
                    for m, (uoff, ucnt) in enumerate(UCH[:2]):
                        acc = pst()
                        for k, (doff, dcnt) in enumerate(DCH):
                            nc.tensor.matmul(
                                acc[:ucnt, :],
                                Wsb[t][k][:dcnt, ds(uoff, ucnt)],
                                xT[t][k][:dcnt, ds(g * NCW, NCW)],
                                start=(k == 0), stop=(k == len(DCH) - 1))
                        if g >= 2:
                            # late groups: ACT is saturated by the affinity
                            # exp wavefront; fused add+max on the idler DVE
                            nc.vector.tensor_scalar(
                                dTP[t][:, m, ds(g * NCW, NCW)], acc[:ucnt, :],
                                bsb[:ucnt, t * 3 + m: t * 3 + m + 1], 0.0,
                                mybir.AluOpType.add, mybir.AluOpType.max)
                        else:
                            nc.scalar.activation(
                                dTP[t][:, m, ds(g * NCW, NCW)], acc[:ucnt, :],
                                AF.Relu,
                                bias=bsb[:ucnt, t * 3 + m: t * 3 + m + 1])
                # the 44-row M-chunk: both tensors' matmuls in concurrent
                # col-groups (0 and 64) of one psum tile
                uoff, ucnt = UCH[2]
                acc2 = pst()
                for k, (doff, dcnt) in enumerate(DCH):
                    fl = (k == 0, k == len(DCH) - 1)
                    nc.tensor.matmul(
                        acc2[0:ucnt, :],
                        Wsb[0][k][:dcnt, ds(uoff, ucnt)],
                        xT[0][k][:dcnt, ds(g * NCW, NCW)],
                        start=fl[0], stop=fl[1], tile_position=(0, 0),
                        skip_group_check=True)
                    nc.tensor.matmul(
                        acc2[64:64 + ucnt, :],
                        Wsb[1][k][:dcnt, ds(uoff, ucnt)],
                        xT[1][k][:dcnt, ds(g * NCW, NCW)],
                        start=fl[0], stop=fl[1], tile_position=(0, 64),
                        skip_group_check=True)
                for t in range(2):
                    pb = t * 64
                    for dst in (0, 64):
                        # evict to base 0 (affinity k2 slice) and base 64
                        # (its row-pair partner slice)
                        if g >= 2:
                            nc.vector.tensor_scalar(
                                dT2[t][dst:dst + ucnt, ds(g * NCW, NCW)],
                                acc2[pb:pb + ucnt, :],
                                bsb[:ucnt, t * 3 + 2: t * 3 + 3], 0.0,
                                mybir.AluOpType.add, mybir.AluOpType.max)
                        else:
                            nc.scalar.activation(
                                dT2[t][dst:dst + ucnt, ds(g * NCW, NCW)],
                                acc2[pb:pb + ucnt, :], AF.Relu,
                                bias=bsb[:ucnt, t * 3 + 2: t * 3 + 3])
                # wavefront affinity: every (row-pair, col-chunk) whose dense
                # inputs just became ready -- spreads the exp (ACT) load into
                # the DMA-paced transpose/dense phase
                for pi2 in range(2 * g, 2 * g + 2):
                    for nx in range(g + 1):
                        aff_chunk(pi2, nx)
                for pi2 in range(0, 2 * g):
                    aff_chunk(pi2, g)
        if TG * 128 != NCW:
            for t in range(2):
                for m, (uoff, ucnt) in enumerate(UCH):
                    for nx in range(NCX):
                        acc = pst()
                        for k, (doff, dcnt) in enumerate(DCH):
                            nc.tensor.matmul(
                                acc[:ucnt, :],
                                Wsb[t][k][:dcnt, ds(uoff, ucnt)],
                                xT[t][k][:dcnt, ds(nx * NCW, NCW)],
                                start=(k == 0), stop=(k == len(DCH) - 1))
                        if m < 2:
                            nc.scalar.activation(
                                dTP[t][:, m, ds(nx * NCW, NCW)], acc[:ucnt, :],
                                AF.Relu,
                                bias=bsb[:ucnt, t * 3 + m: t * 3 + m + 1])
                        else:
                            for dst in (0, 64):
                                nc.scalar.activation(
                                    dT2[t][dst:dst + ucnt, ds(nx * NCW, NCW)],
                                    acc[:ucnt, :], AF.Relu,
                                    bias=bsb[:ucnt, t * 3 + m: t * 3 + m + 1])

        # E2: as each E1 pair tile completes, two xbar DMA transposes copy it
        # (viewed as uint16 q-pairs) into e2all, building exp(A)^T on the idle
        # DMA engines instead of a second affinity+exp pass.
        for pi in range(LT // 2):
            for j in range(2):
                inap = E1[pi][:, j, :].bitcast(u16).rearrange(
                    "p (g u) -> p g u", g=LT // 2)
                pc = (2 * pi + j) * 128
                outap = e2all[:, :, ds(pc, 128), :].bitcast(u16).squeeze()
                nc.sync.dma_start_transpose(outap, inap)

        # helpers ------------------------------------------------------------
        def aligned_T(nats, mov, side_tag, hooks=()):
            """alT tiles [d,L] bf16 = normalized aligned.T, via ones-row trick.

            mov(pi, nsl) yields the fp8 DoubleRow moving AP [128, 2, |nsl|];
            the stationary nats[pi] must pair contraction rows the same way.
            """
            alT = [big.tile([128, L], bf16, tag=f"alT{k}", name=f"alT{side_tag}{k}")
                   for k in range(len(DCH))]
            R = big.tile([128, L], bf16, tag="R", name=f"R{side_tag}")
            NP = LT // 2
            DR = mybir.MatmulPerfMode.DoubleRow
            hooks = list(hooks)
            if hooks:
                hooks.pop(0)()
            # pass A: last d-chunk (88 rows) + ones row at partition 96
            ps4 = [pst() for _ in range(NCX)]
            for pi in range(NP):
                for nx in range(NCX):
                    nc.tensor.matmul(ps4[nx][:, :],
                                     nats[pi][:, :, ds(512, 128)],
                                     mov(pi, ds(nx * NCW, NCW)),
                                     start=(pi == 0), stop=(pi == NP - 1),
                                     perf_mode=DR)
            for nx in range(NCX):
                rrb = rp.tile([128, NCW], bf16, tag="rrb", name="rrb")
                with nc.allow_low_precision(reason="R is consumed as bf16"):
                    nc.vector.reciprocal(rrb[ONES_ROW:ONES_ROW + 1, :],
                                         ps4[nx][ONES_ROW:ONES_ROW + 1, :])
                bc = pst()
                nc.tensor.matmul(bc[:, :], onesb[ONES_ROW:ONES_ROW + 1, 0:128],
                                 rrb[ONES_ROW:ONES_ROW + 1, :],
                                 start=True, stop=True,
                                 tile_position=(ONES_ROW, 0))
                nc.scalar.copy(R[:, ds(nx * NCW, NCW)], bc[:, :])
                nc.vector.tensor_mul(alT[4][0:88, ds(nx * NCW, NCW)],
                                     ps4[nx][0:88, :],
                                     R[0:88, ds(nx * NCW, NCW)])
            # passes B, C: d-chunks 0..3, two at a time
            for mm0 in (0, 2):
                if hooks:
                    hooks.pop(0)()
                accs = {}
                for m in (mm0, mm0 + 1):
                    for nx in range(NCX):
                        accs[(m, nx)] = pst()
                for pi in range(NP):
                    for m in (mm0, mm0 + 1):
                        for nx in range(NCX):
                            nc.tensor.matmul(accs[(m, nx)][:, :],
                                             nats[pi][:, :, ds(m * 128, 128)],
                                             mov(pi, ds(nx * NCW, NCW)),
                                             start=(pi == 0),
                                             stop=(pi == NP - 1),
                                             perf_mode=DR)
                for m in (mm0, mm0 + 1):
                    for nx in range(NCX):
                        nc.vector.tensor_mul(alT[m][:, ds(nx * NCW, NCW)],
                                             accs[(m, nx)][:, :],
                                             R[:, ds(nx * NCW, NCW)])
            while hooks:
                hooks.pop(0)()
            return alT, R

        def fm_proj(s, xTs, bTs):
            """FM projection matmuls for one side; returns live PSUM groups.

            d-chunk-outer loop: the elementwise temps are built full-width once
            per chunk (fewer DVE ops, deeper PE overlap); all four N-chunks'
            projection groups accumulate simultaneously (8 PSUM banks).
            """
            P1s = [ps.tile([128, NCW], f32, tag="ps", name="P1")
                   for _ in range(NCX)]
            P2s = [ps.tile([128, NCW], f32, tag="ps", name="P2")
                   for _ in range(NCX)]
            nk = len(DCH)
            for k, (doff, dcnt) in enumerate(DCH):
                x_fl = xTs[k][:dcnt, :]
                b_fl = bTs[k][:dcnt, :]
                tx2 = fmt.tile([128, L], bf16, tag="fmt", name="tx2")
                tb2 = fmt.tile([128, L], bf16, tag="fmt", name="tb2")
                txm = fmt.tile([128, L], bf16, tag="fmt", name="txm")
                txm2 = fmt.tile([128, L], bf16, tag="fmt", name="txm2")
                # the two independent squares go to ACT (idle through the FM
                # phases); the txm -> txm2 chain stays on the faster DVE
                nc.scalar.activation(tx2[:dcnt, :], x_fl, AF.Square)
                nc.scalar.activation(tb2[:dcnt, :], b_fl, AF.Square)
                nc.vector.tensor_mul(txm[:dcnt, :], x_fl, b_fl)
                nc.vector.tensor_mul(txm2[:dcnt, :], txm[:dcnt, :],
                                     txm[:dcnt, :])
                st = stat[s][k]
                fl = (k == 0, k == nk - 1)
                for nx in range(NCX):
                    nsl = ds(nx * NCW, NCW)
                    P1, P2 = P1s[nx], P2s[nx]
                    nc.tensor.matmul(P1[0:12, :], st[:dcnt, 0:12],
                                     xTs[k][:dcnt, nsl],
                                     start=fl[0], stop=fl[1],
                                     tile_position=(0, 0),
                                     skip_group_check=True)
                    nc.tensor.matmul(P1[32:44, :], st[:dcnt, 12:24],
                                     bTs[k][:dcnt, nsl],
                                     start=fl[0], stop=fl[1],
                                     tile_position=(0, 32),
                                     skip_group_check=True)
                    nc.tensor.matmul(P1[64:65, :], st[:dcnt, 35:36],
                                     txm2[:dcnt, nsl], start=fl[0], stop=fl[1],
                                     tile_position=(0, 64),
                                     skip_group_check=True)
                    nc.tensor.matmul(P2[0:2, :], st[:dcnt, 24:26],
                                     tx2[:dcnt, nsl], start=fl[0], stop=fl[1],
                                     tile_position=(0, 0),
                                     skip_group_check=True)
                    nc.tensor.matmul(P2[32:34, :], st[:dcnt, 26:28],
                                     tb2[:dcnt, nsl], start=fl[0], stop=fl[1],
                                     tile_position=(0, 32),
                                     skip_group_check=True)
                    nc.tensor.matmul(P2[64:71, :], st[:dcnt, 28:35],
                                     txm[:dcnt, nsl], start=fl[0], stop=fl[1],
                                     tile_position=(0, 64),
                                     skip_group_check=True)
            return P1s, P2s

        def fm_comb_build(P1s, P2s, nx, R):
            """S-build (ACT/DVE only) for one N-chunk: evict + square the FM
            groups into S1/S2. Frees the psum banks early so the next
            aligned_T pass's matmuls overlap with the builds."""
            P1, P2 = P1s[nx], P2s[nx]
            S1 = sp.tile([128, NCW], bf16, tag="S1", name="S1", bufs=4)
            S2 = sp.tile([128, NCW], bf16, tag="S2", name="S2", bufs=4)
            nc.gpsimd.memset(S1[:], 0.0)
            nc.gpsimd.memset(S2[:], 0.0)
            # split evictions ACT/DVE so the S-build runs in parallel; all
            # P1 reads first -- they gate the next aligned pass's psum slots
            nc.scalar.copy(S1[0:12, :], P1[0:12, :])
            nc.scalar.copy(S1[32:44, :], P1[32:44, :])
            nc.vector.tensor_copy(S2[32:33, :], P1[64:65, :])
            # B-group Vd columns carry -Vd, so diff quads are also an add.
            TA = sp.tile([10, NCW], f32, tag="TA", name="TA", bufs=2)
            nc.vector.tensor_add(TA[0:10, :], P1[0:10, :], S1[32:42, :])
            nc.vector.tensor_copy(S1[64:66, :], P2[0:2, :])
            nc.vector.tensor_copy(S1[96:98, :], P2[32:34, :])
            nc.vector.tensor_copy(S2[0:7, :], P2[64:71, :])
            nc.scalar.activation(S2[64:74, :], TA[:, :], AF.Square)
            nc.scalar.activation(S2[96:101, :], S2[0:5, :], AF.Square)
            return S1, S2

        def fm_comb_mm(s, Ss, nx):
            """Combine matmuls + bias + output DMA for one N-chunk."""
            S1, S2 = Ss[nx]
            nsl = ds(nx * NCW, NCW)
            cps = ps.tile([3, NCW], f32, tag="ps", name="cps")
            nc.tensor.matmul(cps[:, :], cb2[0:98, 0:3], S1[0:98, :],
                             start=True, stop=False)
            nc.tensor.matmul(cps[:, :], cb2[0:101, 3:6], S2[0:101, :],
                             start=False, stop=True)
            o = ob.tile([3, NCW], f32, tag="ob", name="o")
            nc.scalar.activation(o[:, :], cps[:, :], AF.Identity,
                                 bias=w0sb[:, s:s + 1])
            nc.sync.dma_start(out_d[s, :, nsl], o[:, :])

        # ---------------- main flow ----------------
        qaT, Rq = aligned_T(nats[1],
                            lambda pi, nsl: E1[pi][:, :, nsl], "q")
        P1s, P2s = fm_proj(0, qaT, xT[0])         # passage-side projections
        # S-builds (ACT/DVE) free all 8 psum banks up front; the combine
        # matmuls + outputs interleave with paT's passes so PE never stalls
        Ss0 = [fm_comb_build(P1s, P2s, nx, Rq) for nx in range(NCX)]
        # passage_aligned.T from the DMA-transposed exp(A)^T (adjacent q-pairs)
        paT, Rp = aligned_T(nats[0],
                            lambda pi, nsl: e2all[:, pi, nsl, :].rearrange(
                                "p n s -> p s n"), "p",
                            hooks=[lambda nx=nx: fm_comb_mm(0, Ss0, nx)
                                   for nx in range(NCX)])
        P1s1, P2s1 = fm_proj(1, paT, xT[1])       # query-side projections
        # tail: interleave build/mm per N-chunk so each combine's matmuls
        # start as soon as its own S-build is done (nothing follows to
        # overlap with, so serialization here is pure wall time)
        Ss1 = []
        for nx in range(NCX):
            Ss1.append(fm_comb_build(P1s1, P2s1, nx, Rp))
            fm_comb_mm(1, Ss1, nx)


def _host_prep(W1, b1, W2, b2, cat_w0, cat_w, cat_V, dm_w0, dm_w, dm_V):
    stat = np.zeros((2, D, 36), np.float32)
    for s in range(2):
        ci, di, mi = s, s, s + 2
        Va = cat_V[ci][:, :D]
        Vb = cat_V[ci][:, D:]
        Vd = dm_V[di]
        Vm = dm_V[mi]
        stat[s, :, 0:5] = Va.T
        stat[s, :, 5:10] = Vd.T
        stat[s, :, 10] = cat_w[ci, :D]
        stat[s, :, 11] = dm_w[di]
        stat[s, :, 12:17] = Vb.T
        stat[s, :, 17:22] = -Vd.T   # negated: quad build is then a single add
        stat[s, :, 22] = cat_w[ci, D:]
        stat[s, :, 23] = dm_w[di]
        stat[s, :, 24] = (Va ** 2).sum(0)
        stat[s, :, 25] = (Vd ** 2).sum(0)
        stat[s, :, 26] = (Vb ** 2).sum(0)
        stat[s, :, 27] = (Vd ** 2).sum(0)
        stat[s, :, 28:33] = Vm.T
        stat[s, :, 33] = dm_w[mi]
        stat[s, :, 34] = (Vd ** 2).sum(0)
        stat[s, :, 35] = (Vm ** 2).sum(0)

    # packed combine matrices: S1 = [X@0, B@32, X2@64, B2@96],
    # S2 = [M@0, M2@32, TQ@64, TQM@96]
    comb2 = np.zeros((128, 6), np.float32)
    C1, C2 = comb2[:, 0:3], comb2[:, 3:6]
    C1[10, 0] = 1.0     # x@w_cat -> c_cat
    C1[11, 1] = 1.0     # x@w_d -> c_diff
    C1[32 + 10, 0] = 1.0
    C1[32 + 11, 1] = -1.0
    C1[64, 0] = -0.5    # x2@u_cat
    C1[65, 1] = -0.5    # x2@u_d
    C1[96, 0] = -0.5    # b2@u_cat
    C1[97, 1] = -0.5    # b2@u_d
    C2[5, 2] = 1.0      # mul@w_m
    C2[6, 1] = 1.0      # mul@u_d (from -0.5 * -2)
    C2[32, 2] = -0.5    # mul2@u_m
    C2[64:69, 0] = 0.5  # cat quads
    C2[69:74, 1] = 0.5  # diff quads
    C2[96:101, 2] = 0.5  # mul quads

    # packed per-d-chunk weights / stationaries; the dense bias rides as an
    # extra stationary row (matched by a ones row in xT's last chunk)
    wpack = np.zeros((10, 128, U), np.float32)
    statp = np.zeros((10, 128, 36), np.float32)
    for t, W in enumerate((W1, W2)):
        for k, (doff, dcnt) in enumerate(DCH):
            wpack[t * 5 + k, :dcnt] = W[doff:doff + dcnt]
    for s in range(2):
        for k, (doff, dcnt) in enumerate(DCH):
            statp[s * 5 + k, :dcnt] = stat[s, doff:doff + dcnt]

    biasp = np.zeros((128, 6), np.float32)
    for t, b in enumerate((b1, b2)):
        for m, (uoff, ucnt) in enumerate(UCH):
            biasp[:ucnt, t * 3 + m] = b[uoff:uoff + ucnt]

    w0col = np.zeros((3, 2), np.float32)
    for s in range(2):
        w0col[0, s] = cat_w0[s, 0]
        w0col[1, s] = dm_w0[s, 0]
        w0col[2, s] = dm_w0[s + 2, 0]
    return wpack, statp, comb2, biasp, w0col


_PROG = None


def _get_prog():
    global _PROG
    if _PROG is None:
        from concourse import bacc
        nc = bacc.Bacc(None, target_bir_lowering=False)
        _emit(nc, L_FULL)
        nc.finalize()
        _PROG = nc
    return _PROG


def _in_maps(stack_input, W1, b1, W2, b2, fm_cat_w0, fm_cat_w, fm_cat_V,
             fm_dm_w0, fm_dm_w, fm_dm_V):
    import ml_dtypes
    f = lambda a: np.ascontiguousarray(np.asarray(a, np.float32))
    bf = lambda a: np.ascontiguousarray(np.asarray(a, ml_dtypes.bfloat16))
    stack_input = bf(stack_input)
    wpack, statp, comb2, biasp, w0col = _host_prep(
        f(W1), f(b1), f(W2), f(b2), f(fm_cat_w0), f(fm_cat_w), f(fm_cat_V),
        f(fm_dm_w0), f(fm_dm_w), f(fm_dm_V))
    common = {"wpack": bf(wpack), "statp": bf(statp), "comb2": bf(comb2),
              "biasp": biasp, "w0col": w0col}
    return [dict(common, x=np.ascontiguousarray(stack_input[:, b]))
            for b in range(N_CORES)]


def kernel(stack_input, W1, b1, W2, b2, fm_cat_w0, fm_cat_w, fm_cat_V,
           fm_dm_w0, fm_dm_w, fm_dm_V):
    from concourse.bass_utils import run_bass_kernel_spmd

    in_maps = _in_maps(stack_input, W1, b1, W2, b2, fm_cat_w0, fm_cat_w,
                       fm_cat_V, fm_dm_w0, fm_dm_w, fm_dm_V)
    nc = _get_prog()
    res = run_bass_kernel_spmd(nc, in_maps, core_ids=list(range(N_CORES)))
    outs = [r["out"] for r in res.results]            # each [2, 3, L]
    fp = np.stack([o[0].T for o in outs]).astype(np.float32)   # [8, L, 3]
    fq = np.stack([o[1].T for o in outs]).astype(np.float32)
    return fp, fq



# revision 72
# speedup vs baseline: 1.0054x; 1.0027x over previous
"""Trainium2 Bass kernel for nn_BAC_15152644620305.

Per batch element (1 per NeuronCore, 8 cores):
  p_dense = relu(p @ W1 + b1); q_dense = relu(q @ W2 + b2)
  A = (p_dense @ q_dense.T) / sqrt(600)
  passage_aligned = softmax_rows(A) @ passage ; query_aligned = softmax_cols(A).T @ query
  6 factorization-machine heads on {concat, diff, mul} pairs -> [L, 3] x 2 outputs.

Implementation notes:
  - All heavy matmuls in bf16 (1 cyc/row on PE), fp32 PSUM accumulation;
    aligned/affinity contractions use fp8e4m3 DoubleRow (0.5 cyc/row).
  - Inputs arrive pre-cast to bf16 (halves input DMA, no on-chip f32 casts);
    dense weights/stationaries pre-packed bf16 on the host.
  - Affinity computed ONCE; exp(A) (fp8) is transposed into exp(A)^T by xbar
    DMA transposes (uint16 views batched 8 blocks/instruction) on the
    otherwise-idle DMA engines -- no second affinity+exp pass.  The paT
    contraction consumes the byte-transposed layout via adjacent-(q,q+1)
    DoubleRow pairs; its stationary natural tiles are built adjacent-paired
    directly by a row-interleaved second DMA load of the passage tensor.
  - Affinity is emitted as a wavefront inside the transpose/dense phase:
    each (row-pair, col-chunk) fires as soon as its dense outputs exist,
    spreading the exp (ACT) load into the DMA-paced start.
  - exp without max-subtraction (affinity values are in [0.1, 1.1]).
  - Softmax denominators ride along as an extra ones-column in the aligned
    matmuls' stationary operand, landing at an aligned output partition (96).
  - FM heads algebraically reduced: the x^2 @ V^2.T term needs only
    sum_k V_k^2; diff projections are linear combos of the qa/p projections;
    per-head combination is one small stationary matmul per output chunk.
    The two independent elementwise squares run on ACT (idle during FM),
    the product chain on DVE; S-builds run right after the projections so
    all PSUM banks free before the next aligned pass (its matmuls overlap
    the builds), and the combine matmuls+outputs interleave into that pass.
"""
import numpy as np

L_FULL = 2048
D = 600
U = 300
KFM = 5
N_CORES = 8
SCALE = float(1.0 / np.sqrt(np.float32(D)))

DCH = [(0, 128), (128, 128), (256, 128), (384, 128), (512, 88)]   # D chunks
UCH = [(0, 128), (128, 128), (256, 44)]                           # U chunks
ONES_COL = 608        # column in the 640-wide natural tile holding the ones
ONES_ROW = 96         # output partition where the denominator row lands
NATW = 640


def _emit(nc, L):
    import concourse.bass as bass
    import concourse.mybir as mybir
    import concourse.tile as tile
    from concourse.masks import make_identity
    from contextlib import ExitStack

    f32 = mybir.dt.float32
    bf16 = mybir.dt.bfloat16
    fp8 = mybir.dt.float8e4
    AF = mybir.ActivationFunctionType
    ds = bass.ds

    LT = L // 128               # l tiles
    NCW = min(512, L)           # moving-dim chunk width
    NCX = L // NCW              # chunks per L
    TG = 4 if LT % 4 == 0 else 1  # l-tiles per transpose psum batch

    x_d = nc.dram_tensor("x", [2, L, D], bf16, kind="ExternalInput")
    wp_d = nc.dram_tensor("wpack", [10, 128, U], bf16, kind="ExternalInput")
    sp_d = nc.dram_tensor("statp", [10, 128, 36], bf16, kind="ExternalInput")
    c2_d = nc.dram_tensor("comb2", [128, 6], bf16, kind="ExternalInput")
    bp_d = nc.dram_tensor("biasp", [128, 6], f32, kind="ExternalInput")
    w0_d = nc.dram_tensor("w0col", [3, 2], f32, kind="ExternalInput")
    out_d = nc.dram_tensor("out", [2, 3, L], f32, kind="ExternalOutput")

    u16 = mybir.dt.uint16

    with tile.TileContext(nc) as tc, ExitStack() as ctx:
        const = ctx.enter_context(tc.tile_pool(name="const", bufs=1))
        big = ctx.enter_context(tc.tile_pool(name="big", bufs=1))
        epool = ctx.enter_context(tc.tile_pool(name="epool", bufs=LT // 2))
        natp = ctx.enter_context(tc.tile_pool(name="natp", bufs=LT))
        nf32p = ctx.enter_context(tc.tile_pool(name="nf32p", bufs=6))
        fmt = ctx.enter_context(tc.tile_pool(name="fmt", bufs=4))
        sp = ctx.enter_context(tc.tile_pool(name="sp", bufs=2))
        rp = ctx.enter_context(tc.tile_pool(name="rp", bufs=2))
        ob = ctx.enter_context(tc.tile_pool(name="ob", bufs=1))
        ps = ctx.enter_context(tc.tile_pool(name="ps", bufs=8, space="PSUM"))

        def pst(p_cnt=128, w=NCW):
            return ps.tile([p_cnt, w], f32, tag="ps", name="pst")

        # ------- constants (packed loads on the scalar HWDGE queue) -------
        identb = const.tile([128, 128], bf16, tag="identb")
        make_identity(nc, identb)
        onesb = const.tile([128, 128], bf16, tag="onesb")
        nc.vector.memset(onesb[:], 1.0)
        w0sb = const.tile([3, 2], f32, tag="w0sb")
        nc.scalar.dma_start(w0sb[:], w0_d[:])

        # weights / stationaries arrive pre-packed as bf16: straight DMA loads
        Wall = const.tile([128, 10 * U], bf16, tag="Wall")
        nc.scalar.dma_start(
            Wall[:].rearrange("p (t c) -> p t c", t=10),
            wp_d[:].rearrange("t p c -> p t c"))
        Wsb = [[Wall[:, ds((t * 5 + k) * U, U)] for k in range(5)]
               for t in range(2)]

        Sall = const.tile([128, 360], bf16, tag="Sall")
        nc.scalar.dma_start(
            Sall[:].rearrange("p (t c) -> p t c", t=10),
            sp_d[:].rearrange("t p c -> p t c"))
        stat = [[Sall[:, ds((s * 5 + k) * 36, 36)] for k in range(5)]
                for s in range(2)]

        cb2 = const.tile([128, 6], bf16, tag="cb2")
        nc.scalar.dma_start(cb2[:], c2_d[:])

        bsb = const.tile([128, 6], f32, tag="bsb")
        nc.scalar.dma_start(bsb[:], bp_d[:])

        # ---------------- phase 1: transpose inputs -> pT/qT (bf16 [d, L]) ----
        xT = [[], []]
        for t in range(2):
            for k in range(len(DCH)):
                xT[t].append(big.tile([128, L], bf16, tag=f"xT{t}_{k}",
                                      name=f"xT{t}_{k}"))
        # phase 1+2 interleaved per l-group: transpose inputs -> pT/qT, then
        # the dense matmuls for that group's columns (keeps PE fed during the
        # next group's DMA + cast)
        # u-chunks 0,1 live as one fp8 PAIR tile (DoubleRow operand for the
        # affinity matmuls); the 44-row chunk 2 stays bf16 (base-0 + base-64)
        dTP = [big.tile([128, 2, L], fp8, tag=f"dTP{t}", name=f"dTP{t}")
               for t in range(2)]
        dT2 = [big.tile([128, L], fp8, tag=f"dT2{t}", name=f"dT2{t}")
               for t in range(2)]
        # E2 = byte-transposed E1 (exp(A)^T), written by DMA xbar transposes.
        # Layout [r, g, p, s]: q-row 256*g + 2*r + s, p-col p (adjacent-pair
        # DoubleRow convention over q).
        e2all = big.tile([128, LT // 2, L, 2], fp8, tag="e2all", name="e2all")
        # nats[1] (query): standard pairing (a, j) <-> p-row 256*pi + 128*j + a
        # (matches E1's exp-written slot layout, contraction over p).
        # nats[0] (passage): ADJACENT pairing (r, s) <-> q-row 256*pi + 2*r + s
        # (matches e2all, contraction over q), built from a row-interleaved
        # second load of the passage tensor.
        nats = [[None] * (LT // 2) for _ in range(2)]
        x0i = x_d[0].rearrange("(g r s) d -> g r s d", r=128, s=2)

        DRm = mybir.MatmulPerfMode.DoubleRow
        E1 = [None] * (LT // 2)

        def aff_chunk(pi2, nx):
            """Affinity rows 256*pi2..+256 x cols nx*NCW..+NCW -> exp -> E1."""
            if E1[pi2] is None:
                E1[pi2] = epool.tile([128, 2, L], fp8, tag="E",
                                     name=f"E1_{pi2}")
            e = E1[pi2]
            nsl = ds(nx * NCW, NCW)
            accs = (pst(), pst())
            for j in (0, 1):
                isl = ds((2 * pi2 + j) * 128, 128)
                # u-chunks 0+1 in one fp8 DoubleRow pass
                nc.tensor.matmul(accs[j][:, :], dTP[0][:, :, isl],
                                 dTP[1][:, :, nsl],
                                 start=True, stop=False, perf_mode=DRm)
            # 44-row K chunk: the pair's two matmuls go to disjoint PE
            # row-groups and run concurrently
            nc.tensor.matmul(accs[0][:, :],
                             dT2[0][0:44, ds(2 * pi2 * 128, 128)],
                             dT2[1][0:44, nsl],
                             start=False, stop=True, tile_position=(0, 0))
            nc.tensor.matmul(accs[1][:, :],
                             dT2[0][64:108, ds((2 * pi2 + 1) * 128, 128)],
                             dT2[1][64:108, nsl],
                             start=False, stop=True, tile_position=(64, 0))
            for j in (0, 1):
                nc.scalar.activation(e[:, j, nsl], accs[j][:, :],
                                     AF.Exp, scale=SCALE)

        for g in range(LT // TG):
            gw = TG * 128
            for pi in range(g * TG // 2, (g + 1) * TG // 2):
                nf2 = nf32p.tile([128, 2, D], bf16, tag="nf2", name="nf2",
                                 bufs=1)
                nc.sync.dma_start(nf2[:], x0i[pi])
                nt0 = natp.tile([128, 2, NATW], fp8, tag="nat",
                                name=f"nat0_{pi}")
                nats[0][pi] = nt0
                nc.gpsimd.memset(nt0[:, :, D:NATW], 0.0)
                nc.gpsimd.memset(nt0[:, :, ONES_COL:ONES_COL + 1], 1.0)
                if pi % 2 == 0:
                    nc.vector.tensor_copy(nt0[:, :, 0:D], nf2[:])
                else:
                    nc.gpsimd.tensor_copy(nt0[:, :, 0:D], nf2[:])
            for t in range(2):
                # 2 d-chunks per bf16 psum tile (same 2KB bank footprint as
                # one f32 slot) -> 3 slots instead of 5, more slot headroom
                # for the dense accumulators and the next group's transposes
                pjs2 = [ps.tile([128, 2 * NCW], bf16, tag="ps", name="pjs")
                        for _ in range((len(DCH) + 1) // 2)]
                pjs = [pjs2[k // 2][:, ds((k % 2) * NCW, NCW)]
                       for k in range(len(DCH))]
                for ii in range(TG):
                    i = g * TG + ii
                    nf = nf32p.tile([128, D], bf16, tag="nf", name="nf",
                                    bufs=4)
                    eng = nc.sync if (g == 0 or i % 2 == 0) else nc.scalar
                    eng.dma_start(nf[:], x_d[t, ds(i * 128, 128), :])
                    nfb = nf
                    # build the fp8 natural-layout pair tile (DoubleRow operand
                    # of the aligned matmuls) from the same load; only t=1 --
                    # t=0 is built adjacent-paired from the nf2 loads above
                    pi, j = i // 2, i % 2
                    if t == 1:
                        if j == 0:
                            nats[1][pi] = natp.tile([128, 2, NATW], fp8,
                                                    tag="nat",
                                                    name=f"nat1_{pi}")
                            nc.gpsimd.memset(nats[1][pi][:, :, D:NATW], 0.0)
                            nc.gpsimd.memset(
                                nats[1][pi][:, :, ONES_COL:ONES_COL + 1], 1.0)
                        nt = nats[1][pi]
                        # DVE/Pool: ACT is the bottleneck of this merged phase
                        if j == 0:
                            nc.vector.tensor_copy(nt[:, j, 0:D], nf[:])
                        else:
                            nc.gpsimd.tensor_copy(nt[:, j, 0:D], nf[:])
                    for k, (doff, dcnt) in enumerate(DCH):
                        nc.tensor.transpose(
                            pjs[k][:dcnt, ds(ii * 128, 128)],
                            nfb[:, ds(doff, dcnt)], identb[:])
                for k, (doff, dcnt) in enumerate(DCH):
                    # all on DVE: ACT is the bottleneck of this merged phase
                    nc.vector.tensor_copy(xT[t][k][:dcnt, ds(g * gw, gw)],
                                          pjs[k][:dcnt, ds(0, gw)])
            if gw == NCW:
                for t in range(2):
                    for m, (uoff, ucnt) in enumerate(UCH[:2]):
                        acc = pst()
                        for k, (doff, dcnt) in enumerate(DCH):
                            nc.tensor.matmul(
                                acc[:ucnt, :],
                                Wsb[t][k][:dcnt, ds(uoff, ucnt)],
                                xT[t][k][:dcnt, ds(g * NCW, NCW)],
                                start=(k == 0), stop=(k == len(DCH) - 1))
                        if g >= 2:
                            # late groups: ACT is saturated by the affinity
                            # exp wavefront; fused add+max on the idler DVE
                            nc.vector.tensor_scalar(
                                dTP[t][:, m, ds(g * NCW, NCW)], acc[:ucnt, :],
                                bsb[:ucnt, t * 3 + m: t * 3 + m + 1], 0.0,
                                mybir.AluOpType.add, mybir.AluOpType.max)
                        else:
                            nc.scalar.activation(
                                dTP[t][:, m, ds(g * NCW, NCW)], acc[:ucnt, :],
                                AF.Relu,
                                bias=bsb[:ucnt, t * 3 + m: t * 3 + m + 1])
                # the 44-row M-chunk: both tensors' matmuls in concurrent
                # col-groups (0 and 64) of one psum tile
                uoff, ucnt = UCH[2]
                acc2 = pst()
                for k, (doff, dcnt) in enumerate(DCH):
                    fl = (k == 0, k == len(DCH) - 1)
                    nc.tensor.matmul(
                        acc2[0:ucnt, :],
                        Wsb[0][k][:dcnt, ds(uoff, ucnt)],
                        xT[0][k][:dcnt, ds(g * NCW, NCW)],
                        start=fl[0], stop=fl[1], tile_position=(0, 0),
                        skip_group_check=True)
                    nc.tensor.matmul(
                        acc2[64:64 + ucnt, :],
                        Wsb[1][k][:dcnt, ds(uoff, ucnt)],
                        xT[1][k][:dcnt, ds(g * NCW, NCW)],
                        start=fl[0], stop=fl[1], tile_position=(0, 64),
                        skip_group_check=True)
                for t in range(2):
                    pb = t * 64
                    for dst in (0, 64):
                        # evict to base 0 (affinity k2 slice) and base 64
                        # (its row-pair partner slice)
                        if g >= 2:
                            nc.vector.tensor_scalar(
                                dT2[t][dst:dst + ucnt, ds(g * NCW, NCW)],
                                acc2[pb:pb + ucnt, :],
                                bsb[:ucnt, t * 3 + 2: t * 3 + 3], 0.0,
                                mybir.AluOpType.add, mybir.AluOpType.max)
                        else:
                            nc.scalar.activation(
                                dT2[t][dst:dst + ucnt, ds(g * NCW, NCW)],
                                acc2[pb:pb + ucnt, :], AF.Relu,
                                bias=bsb[:ucnt, t * 3 + 2: t * 3 + 3])
                # wavefront affinity: every (row-pair, col-chunk) whose dense
                # inputs just became ready -- spreads the exp (ACT) load into
                # the DMA-paced transpose/dense phase
                for pi2 in range(2 * g, 2 * g + 2):
                    for nx in range(g + 1):
                        aff_chunk(pi2, nx)
                for pi2 in range(0, 2 * g):
                    aff_chunk(pi2, g)
        if TG * 128 != NCW:
            for t in range(2):
                for m, (uoff, ucnt) in enumerate(UCH):
                    for nx in range(NCX):
                        acc = pst()
                        for k, (doff, dcnt) in enumerate(DCH):
                            nc.tensor.matmul(
                                acc[:ucnt, :],
                                Wsb[t][k][:dcnt, ds(uoff, ucnt)],
                                xT[t][k][:dcnt, ds(nx * NCW, NCW)],
                                start=(k == 0), stop=(k == len(DCH) - 1))
                        if m < 2:
                            nc.scalar.activation(
                                dTP[t][:, m, ds(nx * NCW, NCW)], acc[:ucnt, :],
                                AF.Relu,
                                bias=bsb[:ucnt, t * 3 + m: t * 3 + m + 1])
                        else:
                            for dst in (0, 64):
                                nc.scalar.activation(
                                    dT2[t][dst:dst + ucnt, ds(nx * NCW, NCW)],
                                    acc[:ucnt, :], AF.Relu,
                                    bias=bsb[:ucnt, t * 3 + m: t * 3 + m + 1])

        # E2: as each E1 pair tile completes, two xbar DMA transposes copy it
        # (viewed as uint16 q-pairs) into e2all, building exp(A)^T on the idle
        # DMA engines instead of a second affinity+exp pass.
        for pi in range(LT // 2):
            for j in range(2):
                inap = E1[pi][:, j, :].bitcast(u16).rearrange(
                    "p (g u) -> p g u", g=LT // 2)
                pc = (2 * pi + j) * 128
                outap = e2all[:, :, ds(pc, 128), :].bitcast(u16).squeeze()
                nc.sync.dma_start_transpose(outap, inap)

        # helpers ------------------------------------------------------------
        def aligned_T(nats, mov, side_tag, hooks=()):
            """alT tiles [d,L] bf16 = normalized aligned.T, via ones-row trick.

            mov(pi, nsl) yields the fp8 DoubleRow moving AP [128, 2, |nsl|];
            the stationary nats[pi] must pair contraction rows the same way.
            """
            alT = [big.tile([128, L], bf16, tag=f"alT{k}", name=f"alT{side_tag}{k}")
                   for k in range(len(DCH))]
            R = big.tile([128, L], bf16, tag="R", name=f"R{side_tag}")
            NP = LT // 2
            DR = mybir.MatmulPerfMode.DoubleRow
            hooks = list(hooks)
            if hooks:
                hooks.pop(0)()
            # pass A: last d-chunk (88 rows) + ones row at partition 96
            ps4 = [pst() for _ in range(NCX)]
            for pi in range(NP):
                for nx in range(NCX):
                    nc.tensor.matmul(ps4[nx][:, :],
                                     nats[pi][:, :, ds(512, 128)],
                                     mov(pi, ds(nx * NCW, NCW)),
                                     start=(pi == 0), stop=(pi == NP - 1),
                                     perf_mode=DR)
            for nx in range(NCX):
                rrb = rp.tile([128, NCW], bf16, tag="rrb", name="rrb")
                with nc.allow_low_precision(reason="R is consumed as bf16"):
                    nc.vector.reciprocal(rrb[ONES_ROW:ONES_ROW + 1, :],
                                         ps4[nx][ONES_ROW:ONES_ROW + 1, :])
                bc = pst()
                nc.tensor.matmul(bc[:, :], onesb[ONES_ROW:ONES_ROW + 1, 0:128],
                                 rrb[ONES_ROW:ONES_ROW + 1, :],
                                 start=True, stop=True,
                                 tile_position=(ONES_ROW, 0))
                nc.scalar.copy(R[:, ds(nx * NCW, NCW)], bc[:, :])
                nc.vector.tensor_mul(alT[4][0:88, ds(nx * NCW, NCW)],
                                     ps4[nx][0:88, :],
                                     R[0:88, ds(nx * NCW, NCW)])
            # passes B, C: d-chunks 0..3, two at a time
            for mm0 in (0, 2):
                if hooks:
                    hooks.pop(0)()
                accs = {}
                for m in (mm0, mm0 + 1):
                    for nx in range(NCX):
                        accs[(m, nx)] = pst()
                for pi in range(NP):
                    for m in (mm0, mm0 + 1):
                        for nx in range(NCX):
                            nc.tensor.matmul(accs[(m, nx)][:, :],
                                             nats[pi][:, :, ds(m * 128, 128)],
                                             mov(pi, ds(nx * NCW, NCW)),
                                             start=(pi == 0),
                                             stop=(pi == NP - 1),
                                             perf_mode=DR)
                for m in (mm0, mm0 + 1):
                    for nx in range(NCX):
                        nc.vector.tensor_mul(alT[m][:, ds(nx * NCW, NCW)],
                                             accs[(m, nx)][:, :],
                                             R[:, ds(nx * NCW, NCW)])
            while hooks:
                hooks.pop(0)()
            return alT, R

        def fm_proj(s, xTs, bTs):
            """FM projection matmuls for one side; returns live PSUM groups.

            d-chunk-outer loop: the elementwise temps are built full-width once
            per chunk (fewer DVE ops, deeper PE overlap); all four N-chunks'
            projection groups accumulate simultaneously (8 PSUM banks).
            """
            P1s = [ps.tile([128, NCW], f32, tag="ps", name="P1")
                   for _ in range(NCX)]
            P2s = [ps.tile([128, NCW], f32, tag="ps", name="P2")
                   for _ in range(NCX)]
            nk = len(DCH)
            # chunk 4 first: its aligned slice lands at the end of pass A,
            # so its temps start while the aligned passes B/C still run
            korder = [4, 0, 1, 2, 3]
            for ki, k in enumerate(korder):
                doff, dcnt = DCH[k]
                x_fl = xTs[k][:dcnt, :]
                b_fl = bTs[k][:dcnt, :]
                tx2 = fmt.tile([128, L], bf16, tag="fmt", name="tx2")
                tb2 = fmt.tile([128, L], bf16, tag="fmt", name="tb2")
                txm = fmt.tile([128, L], bf16, tag="fmt", name="txm")
                txm2 = fmt.tile([128, L], bf16, tag="fmt", name="txm2")
                # the two independent squares go to ACT (idle through the FM
                # phases); the txm -> txm2 chain stays on the faster DVE
                nc.scalar.activation(tx2[:dcnt, :], x_fl, AF.Square)
                nc.scalar.activation(tb2[:dcnt, :], b_fl, AF.Square)
                nc.vector.tensor_mul(txm[:dcnt, :], x_fl, b_fl)
                nc.vector.tensor_mul(txm2[:dcnt, :], txm[:dcnt, :],
                                     txm[:dcnt, :])
                st = stat[s][k]
                fl = (ki == 0, ki == nk - 1)
                for nx in range(NCX):
                    nsl = ds(nx * NCW, NCW)
                    P1, P2 = P1s[nx], P2s[nx]
                    nc.tensor.matmul(P1[0:12, :], st[:dcnt, 0:12],
                                     xTs[k][:dcnt, nsl],
                                     start=fl[0], stop=fl[1],
                                     tile_position=(0, 0),
                                     skip_group_check=True)
                    nc.tensor.matmul(P1[32:44, :], st[:dcnt, 12:24],
                                     bTs[k][:dcnt, nsl],
                                     start=fl[0], stop=fl[1],
                                     tile_position=(0, 32),
                                     skip_group_check=True)
                    nc.tensor.matmul(P1[64:65, :], st[:dcnt, 35:36],
                                     txm2[:dcnt, nsl], start=fl[0], stop=fl[1],
                                     tile_position=(0, 64),
                                     skip_group_check=True)
                    nc.tensor.matmul(P2[0:2, :], st[:dcnt, 24:26],
                                     tx2[:dcnt, nsl], start=fl[0], stop=fl[1],
                                     tile_position=(0, 0),
                                     skip_group_check=True)
                    nc.tensor.matmul(P2[32:34, :], st[:dcnt, 26:28],
                                     tb2[:dcnt, nsl], start=fl[0], stop=fl[1],
                                     tile_position=(0, 32),
                                     skip_group_check=True)
                    nc.tensor.matmul(P2[64:71, :], st[:dcnt, 28:35],
                                     txm[:dcnt, nsl], start=fl[0], stop=fl[1],
                                     tile_position=(0, 64),
                                     skip_group_check=True)
            return P1s, P2s

        def fm_comb_build(P1s, P2s, nx, R):
            """S-build (ACT/DVE only) for one N-chunk: evict + square the FM
            groups into S1/S2. Frees the psum banks early so the next
            aligned_T pass's matmuls overlap with the builds."""
            P1, P2 = P1s[nx], P2s[nx]
            S1 = sp.tile([128, NCW], bf16, tag="S1", name="S1", bufs=4)
            S2 = sp.tile([128, NCW], bf16, tag="S2", name="S2", bufs=4)
            nc.gpsimd.memset(S1[:], 0.0)
            nc.gpsimd.memset(S2[:], 0.0)
            # split evictions ACT/DVE so the S-build runs in parallel; all
            # P1 reads first -- they gate the next aligned pass's psum slots
            nc.scalar.copy(S1[0:12, :], P1[0:12, :])
            nc.scalar.copy(S1[32:44, :], P1[32:44, :])
            nc.vector.tensor_copy(S2[32:33, :], P1[64:65, :])
            # B-group Vd columns carry -Vd, so diff quads are also an add.
            TA = sp.tile([10, NCW], f32, tag="TA", name="TA", bufs=2)
            nc.vector.tensor_add(TA[0:10, :], P1[0:10, :], S1[32:42, :])
            nc.vector.tensor_copy(S1[64:66, :], P2[0:2, :])
            nc.vector.tensor_copy(S1[96:98, :], P2[32:34, :])
            nc.vector.tensor_copy(S2[0:7, :], P2[64:71, :])
            nc.scalar.activation(S2[64:74, :], TA[:, :], AF.Square)
            nc.scalar.activation(S2[96:101, :], S2[0:5, :], AF.Square)
            return S1, S2

        def fm_comb_mm(s, Ss, nx):
            """Combine matmuls + bias + output DMA for one N-chunk."""
            S1, S2 = Ss[nx]
            nsl = ds(nx * NCW, NCW)
            cps = ps.tile([3, NCW], f32, tag="ps", name="cps")
            nc.tensor.matmul(cps[:, :], cb2[0:98, 0:3], S1[0:98, :],
                             start=True, stop=False)
            nc.tensor.matmul(cps[:, :], cb2[0:101, 3:6], S2[0:101, :],
                             start=False, stop=True)
            o = ob.tile([3, NCW], f32, tag="ob", name="o")
            nc.scalar.activation(o[:, :], cps[:, :], AF.Identity,
                                 bias=w0sb[:, s:s + 1])
            nc.sync.dma_start(out_d[s, :, nsl], o[:, :])

        # ---------------- main flow ----------------
        qaT, Rq = aligned_T(nats[1],
                            lambda pi, nsl: E1[pi][:, :, nsl], "q")
        P1s, P2s = fm_proj(0, qaT, xT[0])         # passage-side projections
        # S-builds (ACT/DVE) free all 8 psum banks up front; the combine
        # matmuls + outputs interleave with paT's passes so PE never stalls
        Ss0 = [fm_comb_build(P1s, P2s, nx, Rq) for nx in range(NCX)]
        # passage_aligned.T from the DMA-transposed exp(A)^T (adjacent q-pairs)
        paT, Rp = aligned_T(nats[0],
                            lambda pi, nsl: e2all[:, pi, nsl, :].rearrange(
                                "p n s -> p s n"), "p",
                            hooks=[lambda nx=nx: fm_comb_mm(0, Ss0, nx)
                                   for nx in range(NCX)])
        P1s1, P2s1 = fm_proj(1, paT, xT[1])       # query-side projections
        # tail: interleave build/mm per N-chunk so each combine's matmuls
        # start as soon as its own S-build is done (nothing follows to
        # overlap with, so serialization here is pure wall time)
        Ss1 = []
        for nx in range(NCX):
            Ss1.append(fm_comb_build(P1s1, P2s1, nx, Rp))
            fm_comb_mm(1, Ss1, nx)


def _host_prep(W1, b1, W2, b2, cat_w0, cat_w, cat_V, dm_w0, dm_w, dm_V):
    stat = np.zeros((2, D, 36), np.float32)
    for s in range(2):
        ci, di, mi = s, s, s + 2
        Va = cat_V[ci][:, :D]
        Vb = cat_V[ci][:, D:]
        Vd = dm_V[di]
        Vm = dm_V[mi]
        stat[s, :, 0:5] = Va.T
        stat[s, :, 5:10] = Vd.T
        stat[s, :, 10] = cat_w[ci, :D]
        stat[s, :, 11] = dm_w[di]
        stat[s, :, 12:17] = Vb.T
        stat[s, :, 17:22] = -Vd.T   # negated: quad build is then a single add
        stat[s, :, 22] = cat_w[ci, D:]
        stat[s, :, 23] = dm_w[di]
        stat[s, :, 24] = (Va ** 2).sum(0)
        stat[s, :, 25] = (Vd ** 2).sum(0)
        stat[s, :, 26] = (Vb ** 2).sum(0)
        stat[s, :, 27] = (Vd ** 2).sum(0)
        stat[s, :, 28:33] = Vm.T
        stat[s, :, 33] = dm_w[mi]
        stat[s, :, 34] = (Vd ** 2).sum(0)
        stat[s, :, 35] = (Vm ** 2).sum(0)

    # packed combine matrices: S1 = [X@0, B@32, X2@64, B2@96],
    # S2 = [M@0, M2@32, TQ@64, TQM@96]
    comb2 = np.zeros((128, 6), np.float32)
    C1, C2 = comb2[:, 0:3], comb2[:, 3:6]
    C1[10, 0] = 1.0     # x@w_cat -> c_cat
    C1[11, 1] = 1.0     # x@w_d -> c_diff
    C1[32 + 10, 0] = 1.0
    C1[32 + 11, 1] = -1.0
    C1[64, 0] = -0.5    # x2@u_cat
    C1[65, 1] = -0.5    # x2@u_d
    C1[96, 0] = -0.5    # b2@u_cat
    C1[97, 1] = -0.5    # b2@u_d
    C2[5, 2] = 1.0      # mul@w_m
    C2[6, 1] = 1.0      # mul@u_d (from -0.5 * -2)
    C2[32, 2] = -0.5    # mul2@u_m
    C2[64:69, 0] = 0.5  # cat quads
    C2[69:74, 1] = 0.5  # diff quads
    C2[96:101, 2] = 0.5  # mul quads

    # packed per-d-chunk weights / stationaries; the dense bias rides as an
    # extra stationary row (matched by a ones row in xT's last chunk)
    wpack = np.zeros((10, 128, U), np.float32)
    statp = np.zeros((10, 128, 36), np.float32)
    for t, W in enumerate((W1, W2)):
        for k, (doff, dcnt) in enumerate(DCH):
            wpack[t * 5 + k, :dcnt] = W[doff:doff + dcnt]
    for s in range(2):
        for k, (doff, dcnt) in enumerate(DCH):
            statp[s * 5 + k, :dcnt] = stat[s, doff:doff + dcnt]

    biasp = np.zeros((128, 6), np.float32)
    for t, b in enumerate((b1, b2)):
        for m, (uoff, ucnt) in enumerate(UCH):
            biasp[:ucnt, t * 3 + m] = b[uoff:uoff + ucnt]

    w0col = np.zeros((3, 2), np.float32)
    for s in range(2):
        w0col[0, s] = cat_w0[s, 0]
        w0col[1, s] = dm_w0[s, 0]
        w0col[2, s] = dm_w0[s + 2, 0]
    return wpack, statp, comb2, biasp, w0col


_PROG = None


def _get_prog():
    global _PROG
    if _PROG is None:
        from concourse import bacc
        nc = bacc.Bacc(None, target_bir_lowering=False)
        _emit(nc, L_FULL)
        nc.finalize()
        _PROG = nc
    return _PROG


def _in_maps(stack_input, W1, b1, W2, b2, fm_cat_w0, fm_cat_w, fm_cat_V,
             fm_dm_w0, fm_dm_w, fm_dm_V):
    import ml_dtypes
    f = lambda a: np.ascontiguousarray(np.asarray(a, np.float32))
    bf = lambda a: np.ascontiguousarray(np.asarray(a, ml_dtypes.bfloat16))
    stack_input = bf(stack_input)
    wpack, statp, comb2, biasp, w0col = _host_prep(
        f(W1), f(b1), f(W2), f(b2), f(fm_cat_w0), f(fm_cat_w), f(fm_cat_V),
        f(fm_dm_w0), f(fm_dm_w), f(fm_dm_V))
    common = {"wpack": bf(wpack), "statp": bf(statp), "comb2": bf(comb2),
              "biasp": biasp, "w0col": w0col}
    return [dict(common, x=np.ascontiguousarray(stack_input[:, b]))
            for b in range(N_CORES)]


def kernel(stack_input, W1, b1, W2, b2, fm_cat_w0, fm_cat_w, fm_cat_V,
           fm_dm_w0, fm_dm_w, fm_dm_V):
    from concourse.bass_utils import run_bass_kernel_spmd

    in_maps = _in_maps(stack_input, W1, b1, W2, b2, fm_cat_w0, fm_cat_w,
                       fm_cat_V, fm_dm_w0, fm_dm_w, fm_dm_V)
    nc = _get_prog()
    res = run_bass_kernel_spmd(nc, in_maps, core_ids=list(range(N_CORES)))
    outs = [r["out"] for r in res.results]            # each [2, 3, L]
    fp = np.stack([o[0].T for o in outs]).astype(np.float32)   # [8, L, 3]
    fq = np.stack([o[1].T for o in outs]).astype(np.float32)
    return fp, fq

